# revision 1
# baseline (speedup 1.0000x reference)
"""Trainium2 Bass kernel for a Griffin-style ChimeraBlock:
   pre-norm RG-LRU recurrence branch + pre-norm SwiGLU FFN, B=2, T=2048,
   D=H=2048, FFN=5632, fp32 I/O.

Parallelization over 8 NeuronCores (tensor-parallel):
  - recurrence hidden dim H sharded 8x (256 per core); scan is elementwise
    per channel -> shards cleanly (native DVE tensor_tensor_scan op).
  - hs rmsnorm: partial sum-of-squares + tiny AllReduce.
  - AllGather of normalized hs (bf16) -> rec_out computed column-parallel
    (each core produces a D-shard of rec_out; no big reduce needed).
  - FFN hidden sharded 8x (704 -> padded 768 per core); down-proj partials
    ReduceScattered over D; each core emits its D-shard of the final output.
Matmuls run in bf16 (fp32 accumulation in PSUM); norms/scan state in fp32;
residual path keeps x in fp32.
Host side only reshapes/transposes/casts/shards and folds the (all-ones)
rmsnorm gain vectors into adjacent weight matrices.
"""

import sys

sys.path.insert(0, "/opt/trn_rl_repo")

import numpy as np
import ml_dtypes

import concourse.bass as bass
import concourse.mybir as mybir
import concourse.tile as tile
from concourse import bacc
from concourse.bass_utils import run_bass_kernel_spmd

BF16 = mybir.dt.bfloat16
F32 = mybir.dt.float32
AF = mybir.ActivationFunctionType
OP = mybir.AluOpType

B, T, D = 2, 2048, 2048
H, FFN = 2048, 5632
NC = 8
HS = H // NC          # 256 hidden shard
DS = D // NC          # 256 d-model shard (output sharding)
FS = FFN // NC        # 704 ffn shard
FSP = 768             # ffn shard padded to a multiple of 128 (pad weights = 0)
BT = B * T            # 4096
CH = 512              # time-chunk (columns)
NCH = BT // CH        # 8 chunks
CPB = T // CH         # 4 chunks per batch element (scan resets at b boundary)
KD = D // 128         # 16 k-tiles when contracting over D
KH = H // 128         # 16 k-tiles when contracting over H
KF = FSP // 128       # 6 k-tiles when contracting over ffn shard
EPS = 1e-6
CCONST = 8.0

NP_BF16 = ml_dtypes.bfloat16


def _r128(ap):
    # [R, N] dram view -> [128, R//128, N] (partition, k-tile, col)
    return ap.rearrange("(k p) n -> p k n", p=128)


def build_nc(phases=7, repeat=1):
    nc = bacc.Bacc("TRN2", target_bir_lowering=False, debug=False, num_devices=NC)
    rg = [list(range(NC))]

    # ---------------- kernel I/O (per core) ----------------
    xt = nc.dram_tensor("xt", [D, BT], BF16, kind="ExternalInput")      # x^T replicated
    xf32 = nc.dram_tensor("xf32", [DS, BT], F32, kind="ExternalInput")  # f32 x^T d-shard
    w3 = nc.dram_tensor("w3", [D, 3 * HS], BF16, kind="ExternalInput")  # in|ig|rg lhsT shard
    wro = nc.dram_tensor("wro", [H, DS], BF16, kind="ExternalInput")    # rec_out lhsT d-shard
    wg = nc.dram_tensor("wg", [D, FSP], BF16, kind="ExternalInput")
    wu = nc.dram_tensor("wu", [D, FSP], BF16, kind="ExternalInput")
    wd = nc.dram_tensor("wd", [FSP, D], BF16, kind="ExternalInput")
    # cols: 0 = rec_lambda, 1 = ig bias, 2 = rg bias, 3 = h0
    smalls = nc.dram_tensor("smalls", [HS, 4], F32, kind="ExternalInput")
    y = nc.dram_tensor("y", [DS, BT], F32, kind="ExternalOutput")

    with tile.TileContext(nc) as tc:
        with (
            tc.tile_pool(name="sb", bufs=2) as sb,
            tc.tile_pool(name="ps", bufs=2, space="PSUM") as ps,
            tc.tile_pool(name="dr", bufs=1, space="DRAM") as dr,
        ):
            for _rep in range(repeat):
                build_body(nc, tc, sb, ps, dr, rg,
                           xt, xf32, w3, wro, wg, wu, wd, smalls, y, phases)
    nc.compile()
    return nc


def build_body(nc, tc, sb, ps, dr, rg, xt, xf32, w3, wro, wg, wu, wd, smalls, y, phases=7):
    def finish_early():
        fin = sb.tile([128, 4], F32, name="fin", tag="fin", bufs=1)
        nc.vector.memset(fin[:], 0.0)
        nc.sync.dma_start(out=y[0:128, 0:4], in_=fin[:])
    AG = "AllGather"
    AR = "AllReduce"
    RS = "ReduceScatter"

    # ---------------- internal DRAM ----------------
    ar1_in = dr.tile([1, BT], F32, name="ar1_in")
    ar1_out = dr.tile([1, BT], F32, name="ar1_out", addr_space="Shared")
    ar2_in = dr.tile([1, BT], F32, name="ar2_in")
    ar2_out = dr.tile([1, BT], F32, name="ar2_out", addr_space="Shared")
    ar3_in = dr.tile([1, BT], F32, name="ar3_in")
    ar3_out = dr.tile([1, BT], F32, name="ar3_out", addr_space="Shared")
    xnew_dram = dr.tile([DS, BT], F32, name="xnew_dram")
    agin_hs = [dr.tile([HS, 2 * CH], BF16, name=f"agin_hs{j}") for j in range(4)]
    agout_hs = [dr.tile([H, 2 * CH], BF16, name=f"agout_hs{j}", addr_space="Shared")
                for j in range(4)]
    agin_h2 = [dr.tile([DS, 2 * CH], BF16, name=f"agin_h2{j}") for j in range(4)]
    agout_h2 = [dr.tile([D, 2 * CH], BF16, name=f"agout_h2{j}", addr_space="Shared")
                for j in range(4)]
    ffn_part = [dr.tile([D, 2 * CH], BF16, name=f"ffn_part{j}") for j in range(4)]
    ffn_red = [dr.tile([DS, 2 * CH], BF16, name=f"ffn_red{j}") for j in range(4)]

    dma = nc.sync.dma_start

    # ---------------- constants / small tensors ----------------
    ones_bf = sb.tile([128, 1], BF16, name="ones_bf", tag="ones", bufs=1)
    nc.vector.memset(ones_bf[:], 1.0)

    def const_tile(val, cname):
        t = sb.tile([128, 1], F32, name=cname, tag=cname, bufs=1)
        nc.vector.memset(t[:], val)
        return t

    c_ln8 = const_tile(1e-8, "c_ln8")       # Ln bias
    c_eps = const_tile(EPS, "c_eps")        # rmsnorm eps
    c_1eps = const_tile(1.0 + EPS, "c_1eps")  # 1 + eps for sqrt(1 - a^2 + eps)

    smalls_sb = sb.tile([128, 2, 4], F32, name="smalls_sb", tag="smalls", bufs=1)
    dma(out=smalls_sb[:], in_=smalls[:].rearrange("(a p) c -> p a c", p=128))
    sig_l = sb.tile([128, 2], F32, name="sig_l", tag="sig_l", bufs=1)
    nc.scalar.activation(sig_l[:], smalls_sb[:, :, 0], AF.Sigmoid)
    c8_sb = sb.tile([128, 2], F32, name="c8_sb", tag="c8", bufs=1)
    # log(sigmoid(lambda) + 1e-8)
    nc.scalar.activation(c8_sb[:], sig_l[:], AF.Ln, bias=c_ln8[:])
    # * C (in place via Copy with scale)
    nc.scalar.activation(c8_sb[:], c8_sb[:], AF.Copy, bias=0.0, scale=CCONST)

    # ---------------- weights ----------------
    w3_sb = sb.tile([128, KD, 3 * HS], BF16, name="w3_sb", tag="bigw", bufs=3)
    dma(out=w3_sb[:], in_=_r128(w3[:]))
    wro_sb = sb.tile([128, KH, DS], BF16, name="wro_sb", tag="wro", bufs=1)
    dma(out=wro_sb[:], in_=_r128(wro[:]))

    # ---------------- phase 1: norm1 stats (partial over d-shard) ----------------
    for c in range(NCH):
        cs = slice(c * CH, (c + 1) * CH)
        xft = sb.tile([128, 2, CH], F32, name=f"xft{c}", tag="xf", bufs=2)
        dma(out=xft[:], in_=_r128(xf32[:])[:, :, cs])
        xsq = sb.tile([128, 2, CH], BF16, name=f"xsq{c}", tag="bfa", bufs=2)
        nc.scalar.activation(xsq[:], xft[:], AF.Square)
        psq = ps.tile([1, CH], F32, name=f"psq1_{c}", tag="psq", bufs=2)
        nc.tensor.matmul(psq[:], ones_bf[:], xsq[:, 0, :], start=True, stop=False)
        nc.tensor.matmul(psq[:], ones_bf[:], xsq[:, 1, :], start=False, stop=True)
        sqs = sb.tile([1, CH], F32, name=f"sqs1_{c}", tag="sqs", bufs=2)
        nc.scalar.copy(sqs[:], psq[:])
        dma(out=ar1_in[0:1, cs], in_=sqs[:])
    nc.gpsimd.collective_compute(AR, OP.add, replica_groups=rg,
                                 ins=[ar1_in[:]], outs=[ar1_out[:]])

    if phases < 2:
        finish_early()
        return
    # ---------------- phase 2: in-proj + gates + scan ----------------
    hst_prev = None
    for c in range(NCH):
        cs = slice(c * CH, (c + 1) * CH)
        # inv_rms1 for this chunk, replicated across partitions
        arc = sb.tile([1, CH], F32, name=f"arc1_{c}", tag="arc", bufs=2)
        dma(out=arc[:], in_=ar1_out[0:1, cs])
        nc.scalar.activation(arc[:], arc[:], AF.Sqrt, bias=c_eps[:1, :], scale=1.0 / D)
        nc.vector.reciprocal(arc[:], arc[:])
        invc = sb.tile([128, CH], F32, name=f"invc1_{c}", tag="invc", bufs=2)
        nc.gpsimd.partition_broadcast(invc[:], arc[:])

        xc = sb.tile([128, KD, CH], BF16, name=f"xc{c}", tag="stream", bufs=2)
        dma(out=xc[:], in_=_r128(xt[:])[:, :, cs])

        zt = {}
        for p_i in range(3):  # 0: x_proj, 1: input gate, 2: recurrence gate
            for m in range(2):
                pst = ps.tile([128, CH], F32, name=f"pp{c}_{p_i}_{m}", tag="mm", bufs=6)
                for k in range(KD):
                    nc.tensor.matmul(
                        pst[:],
                        w3_sb[:, k, p_i * HS + m * 128: p_i * HS + (m + 1) * 128],
                        xc[:, k, :],
                        start=(k == 0), stop=(k == KD - 1),
                    )
                z = sb.tile([128, CH], F32, name=f"z{c}_{p_i}_{m}",
                            tag=f"z{p_i}", bufs=2)
                nc.vector.tensor_tensor(z[:], pst[:], invc[:], op=OP.mult)
                zt[(p_i, m)] = z

        hst = sb.tile([128, 2, CH], BF16, name=f"hst{c}", tag="hs", bufs=3)
        for m in range(2):
            zx, zi, zr = zt[(0, m)], zt[(1, m)], zt[(2, m)]
            it = sb.tile([128, CH], F32, name=f"it{c}_{m}", tag="it", bufs=2)
            nc.scalar.activation(it[:], zi[:], AF.Sigmoid,
                                 bias=smalls_sb[:, m, 1:2])
            rt = sb.tile([128, CH], F32, name=f"rt{c}_{m}", tag="rt", bufs=2)
            nc.scalar.activation(rt[:], zr[:], AF.Sigmoid,
                                 bias=smalls_sb[:, m, 2:3])
            # la = r * (C * log_a)  (in place over rt)
            nc.vector.tensor_scalar_mul(rt[:], rt[:], c8_sb[:, m:m + 1])
            at = sb.tile([128, CH], F32, name=f"at{c}_{m}", tag="at", bufs=2)
            nc.scalar.activation(at[:], rt[:], AF.Exp)
            # na = -a^2 ; sq = sqrt(1 + eps - a^2)
            na = sb.tile([128, CH], F32, name=f"na{c}_{m}", tag="na", bufs=2)
            nc.vector.scalar_tensor_tensor(na[:], at[:], -1.0, at[:],
                                           op0=OP.mult, op1=OP.mult)
            nc.scalar.activation(na[:], na[:], AF.Sqrt, bias=c_1eps[:])
            # g = sq * (i * x_proj)   (build in place over zx)
            nc.vector.tensor_tensor(zx[:], it[:], zx[:], op=OP.mult)
            nc.vector.tensor_tensor(zx[:], na[:], zx[:], op=OP.mult)
            if c % CPB == 0:
                init = smalls_sb[:, m, 3:4]
            else:
                init = hst_prev[:, m, CH - 1:CH]
            nc.vector.tensor_tensor_scan(hst[:, m, :], at[:], zx[:], init,
                                         op0=OP.mult, op1=OP.add)
        hst_prev = hst

        # partial sumsq of hs over the h-shard
        hsq = sb.tile([128, 2, CH], BF16, name=f"hsq{c}", tag="bfb", bufs=2)
        nc.scalar.activation(hsq[:], hst[:], AF.Square)
        psq2 = ps.tile([1, CH], F32, name=f"psq2_{c}", tag="psq", bufs=2)
        nc.tensor.matmul(psq2[:], ones_bf[:], hsq[:, 0, :], start=True, stop=False)
        nc.tensor.matmul(psq2[:], ones_bf[:], hsq[:, 1, :], start=False, stop=True)
        sqs2 = sb.tile([1, CH], F32, name=f"sqs2_{c}", tag="sqs", bufs=2)
        nc.scalar.copy(sqs2[:], psq2[:])
        dma(out=ar2_in[0:1, cs], in_=sqs2[:])
        j, jj = c // 2, c % 2
        dma(out=_r128(agin_hs[j][:])[:, :, jj * CH:(jj + 1) * CH], in_=hst[:])
        if jj == 1:
            nc.gpsimd.collective_compute(AG, OP.bypass, replica_groups=rg,
                                         ins=[agin_hs[j][:]], outs=[agout_hs[j][:]])

    nc.gpsimd.collective_compute(AR, OP.add, replica_groups=rg,
                                 ins=[ar2_in[:]], outs=[ar2_out[:]])

    if phases < 4:
        finish_early()
        return
    # ---------------- phase 4: rec_out (d-shard) + residual + norm2 stats ----------------
    for c in range(NCH):
        cs = slice(c * CH, (c + 1) * CH)
        j, jj = c // 2, c % 2
        arc = sb.tile([1, CH], F32, name=f"arc2_{c}", tag="arc", bufs=2)
        dma(out=arc[:], in_=ar2_out[0:1, cs])
        nc.scalar.activation(arc[:], arc[:], AF.Sqrt, bias=c_eps[:1, :], scale=1.0 / H)
        nc.vector.reciprocal(arc[:], arc[:])
        invc = sb.tile([128, CH], F32, name=f"invc2_{c}", tag="invc", bufs=2)
        nc.gpsimd.partition_broadcast(invc[:], arc[:])
        hstm = sb.tile([128, KH, CH], BF16, name=f"hstm{c}", tag="stream", bufs=2)
        dma(out=hstm[:], in_=_r128(agout_hs[j][:])[:, :, jj * CH:(jj + 1) * CH])
        xft = sb.tile([128, 2, CH], F32, name=f"xfr{c}", tag="xf", bufs=2)
        dma(out=xft[:], in_=_r128(xf32[:])[:, :, cs])
        xnt = sb.tile([128, 2, CH], F32, name=f"xnt{c}", tag="xn", bufs=2)
        for m in range(2):
            pst = ps.tile([128, CH], F32, name=f"pro{c}_{m}", tag="mm", bufs=6)
            for k in range(KH):
                nc.tensor.matmul(pst[:], wro_sb[:, k, m * 128:(m + 1) * 128],
                                 hstm[:, k, :],
                                 start=(k == 0), stop=(k == KH - 1))
            ro = sb.tile([128, CH], F32, name=f"ro{c}_{m}", tag="roz", bufs=2)
            nc.vector.tensor_tensor(ro[:], pst[:], invc[:], op=OP.mult)
            nc.vector.tensor_tensor(xnt[:, m, :], ro[:], xft[:, m, :], op=OP.add)
        xnq = sb.tile([128, 2, CH], BF16, name=f"xnq{c}", tag="bfa", bufs=2)
        nc.scalar.activation(xnq[:], xnt[:], AF.Square)
        psq3 = ps.tile([1, CH], F32, name=f"psq3_{c}", tag="psq", bufs=2)
        nc.tensor.matmul(psq3[:], ones_bf[:], xnq[:, 0, :], start=True, stop=False)
        nc.tensor.matmul(psq3[:], ones_bf[:], xnq[:, 1, :], start=False, stop=True)
        sqs3 = sb.tile([1, CH], F32, name=f"sqs3_{c}", tag="sqs", bufs=2)
        nc.scalar.copy(sqs3[:], psq3[:])
        dma(out=ar3_in[0:1, cs], in_=sqs3[:])
        dma(out=_r128(xnew_dram[:])[:, :, cs], in_=xnt[:])

    nc.gpsimd.collective_compute(AR, OP.add, replica_groups=rg,
                                 ins=[ar3_in[:]], outs=[ar3_out[:]])

    if phases < 5:
        finish_early()
        return
    # ---------------- phase 5: h2 = rmsnorm(x_new), AllGather ----------------
    for c in range(NCH):
        cs = slice(c * CH, (c + 1) * CH)
        arc = sb.tile([1, CH], F32, name=f"arc3_{c}", tag="arc", bufs=2)
        dma(out=arc[:], in_=ar3_out[0:1, cs])
        nc.scalar.activation(arc[:], arc[:], AF.Sqrt, bias=c_eps[:1, :], scale=1.0 / D)
        nc.vector.reciprocal(arc[:], arc[:])
        invc = sb.tile([128, CH], F32, name=f"invc3_{c}", tag="invc", bufs=2)
        nc.gpsimd.partition_broadcast(invc[:], arc[:])

        xnt2 = sb.tile([128, 2, CH], F32, name=f"xnt2_{c}", tag="xf", bufs=2)
        dma(out=xnt2[:], in_=_r128(xnew_dram[:])[:, :, cs])
        h2t = sb.tile([128, 2, CH], BF16, name=f"h2t{c}", tag="bfa", bufs=2)
        for m in range(2):
            nc.vector.tensor_tensor(h2t[:, m, :], xnt2[:, m, :], invc[:], op=OP.mult)
        j, jj = c // 2, c % 2
        dma(out=_r128(agin_h2[j][:])[:, :, jj * CH:(jj + 1) * CH], in_=h2t[:])
        if jj == 1:
            nc.gpsimd.collective_compute(AG, OP.bypass, replica_groups=rg,
                                         ins=[agin_h2[j][:]], outs=[agout_h2[j][:]])

    if phases < 6:
        finish_early()
        return
    # ---------------- phase 6: FFN ----------------
    wg_sb = sb.tile([128, KD, FSP], BF16, name="wg_sb", tag="bigw", bufs=3)
    dma(out=wg_sb[:], in_=_r128(wg[:]))
    wu_sb = sb.tile([128, KD, FSP], BF16, name="wu_sb", tag="bigw", bufs=3)
    dma(out=wu_sb[:], in_=_r128(wu[:]))
    wd_sb = sb.tile([128, KF, D], BF16, name="wd_sb", tag="bigw", bufs=3)
    dma(out=wd_sb[:], in_=_r128(wd[:]))

    for c in range(NCH):
        j, jj = c // 2, c % 2
        h2s = sb.tile([128, KD, CH], BF16, name=f"h2s{c}", tag="stream", bufs=2)
        dma(out=h2s[:], in_=_r128(agout_h2[j][:])[:, :, jj * CH:(jj + 1) * CH])
        gu = sb.tile([128, KF, CH], BF16, name=f"gu{c}", tag="gu", bufs=2)
        for m in range(KF):
            psg = ps.tile([128, CH], F32, name=f"pg{c}_{m}", tag="mm", bufs=6)
            for k in range(KD):
                nc.tensor.matmul(psg[:], wg_sb[:, k, m * 128:(m + 1) * 128],
                                 h2s[:, k, :],
                                 start=(k == 0), stop=(k == KD - 1))
            gs = sb.tile([128, CH], BF16, name=f"gs{c}_{m}", tag="gsil", bufs=2)
            nc.scalar.activation(gs[:], psg[:], AF.Silu)
            psu = ps.tile([128, CH], F32, name=f"pu{c}_{m}", tag="mm", bufs=6)
            for k in range(KD):
                nc.tensor.matmul(psu[:], wu_sb[:, k, m * 128:(m + 1) * 128],
                                 h2s[:, k, :],
                                 start=(k == 0), stop=(k == KD - 1))
            nc.vector.tensor_tensor(gu[:, m, :], psu[:], gs[:], op=OP.mult)
        for m in range(KD):
            psd = ps.tile([128, CH], F32, name=f"pd{c}_{m}", tag="mm", bufs=6)
            for k in range(KF):
                nc.tensor.matmul(psd[:], wd_sb[:, k, m * 128:(m + 1) * 128],
                                 gu[:, k, :],
                                 start=(k == 0), stop=(k == KF - 1))
            dst = sb.tile([128, CH], BF16, name=f"dst{c}_{m}", tag="dstage", bufs=3)
            nc.any.tensor_copy(dst[:], psd[:])
            dma(out=ffn_part[j][m * 128:(m + 1) * 128, jj * CH:(jj + 1) * CH],
                in_=dst[:])
        if jj == 1:
            nc.gpsimd.collective_compute(RS, OP.add, replica_groups=rg,
                                         ins=[ffn_part[j][:]], outs=[ffn_red[j][:]])

    if phases < 7:
        finish_early()
        return
    # ---------------- phase 7: final residual ----------------
    for c in range(NCH):
        cs = slice(c * CH, (c + 1) * CH)
        j, jj = c // 2, c % 2
        frt = sb.tile([128, 2, CH], BF16, name=f"frt{c}", tag="bfb", bufs=2)
        dma(out=frt[:], in_=_r128(ffn_red[j][:])[:, :, jj * CH:(jj + 1) * CH])
        xnt3 = sb.tile([128, 2, CH], F32, name=f"xnt3_{c}", tag="xf", bufs=2)
        dma(out=xnt3[:], in_=_r128(xnew_dram[:])[:, :, cs])
        yt = sb.tile([128, 2, CH], F32, name=f"yt{c}", tag="xn", bufs=2)
        for m in range(2):
            nc.vector.tensor_tensor(yt[:, m, :], xnt3[:, m, :], frt[:, m, :],
                                    op=OP.add)
        dma(out=_r128(y[:])[:, :, cs], in_=yt[:])


_CACHE = {}


def _prep_inputs(inputs):
    f = np.float32
    x = np.asarray(inputs["x"], f)                       # [B, T, D]
    norm1_w = np.asarray(inputs["norm1_w"], f)
    rec_in_w = np.asarray(inputs["rec_in_w"], f)         # [H, D]
    rec_ig_w = np.asarray(inputs["rec_ig_w"], f)
    rec_ig_b = np.asarray(inputs["rec_ig_b"], f)
    rec_rg_w = np.asarray(inputs["rec_rg_w"], f)
    rec_rg_b = np.asarray(inputs["rec_rg_b"], f)
    rec_lambda = np.asarray(inputs["rec_lambda"], f)
    rec_out_w = np.asarray(inputs["rec_out_w"], f)       # [D, H]
    rec_h0 = np.asarray(inputs["rec_h0"], f)             # [1, 1, H]
    rec_norm_w = np.asarray(inputs["rec_norm_w"], f)
    norm2_w = np.asarray(inputs["norm2_w"], f)
    ffn_gate_w = np.asarray(inputs["ffn_gate_w"], f)     # [FFN, D]
    ffn_up_w = np.asarray(inputs["ffn_up_w"], f)
    ffn_down_w = np.asarray(inputs["ffn_down_w"], f)     # [D, FFN]

    xt_full = np.ascontiguousarray(
        x.reshape(BT, D).T.astype(NP_BF16))              # [D, BT]
    xt_f32 = np.ascontiguousarray(x.reshape(BT, D).T)    # [D, BT] f32

    # fold norm gains into adjacent weights; transpose into lhsT layouts
    w_in_t = (rec_in_w * norm1_w[None, :]).T             # [D, H]
    w_ig_t = (rec_ig_w * norm1_w[None, :]).T
    w_rg_t = (rec_rg_w * norm1_w[None, :]).T
    w_ro_t = (rec_out_w * rec_norm_w[None, :]).T         # [H, D]
    w_g_t = (ffn_gate_w * norm2_w[None, :]).T            # [D, FFN]
    w_u_t = (ffn_up_w * norm2_w[None, :]).T
    w_d_t = ffn_down_w.T                                 # [FFN, D]

    in_maps = []
    for r in range(NC):
        hsl = slice(r * HS, (r + 1) * HS)
        dsl = slice(r * DS, (r + 1) * DS)
        fsl = slice(r * FS, (r + 1) * FS)
        w3_r = np.concatenate(
            [w_in_t[:, hsl], w_ig_t[:, hsl], w_rg_t[:, hsl]], axis=1)
        wg_r = np.zeros((D, FSP), f)
        wg_r[:, :FS] = w_g_t[:, fsl]
        wu_r = np.zeros((D, FSP), f)
        wu_r[:, :FS] = w_u_t[:, fsl]
        wd_r = np.zeros((FSP, D), f)
        wd_r[:FS, :] = w_d_t[fsl, :]
        smalls_r = np.stack(
            [rec_lambda[hsl], rec_ig_b[hsl], rec_rg_b[hsl],
             np.broadcast_to(rec_h0[0, 0], (H,))[hsl]], axis=1)
        in_maps.append({
            "xt": xt_full,
            "xf32": np.ascontiguousarray(xt_f32[dsl, :]),
            "w3": np.ascontiguousarray(w3_r.astype(NP_BF16)),
            "wro": np.ascontiguousarray(w_ro_t[:, dsl].astype(NP_BF16)),
            "wg": np.ascontiguousarray(wg_r.astype(NP_BF16)),
            "wu": np.ascontiguousarray(wu_r.astype(NP_BF16)),
            "wd": np.ascontiguousarray(wd_r.astype(NP_BF16)),
            "smalls": np.ascontiguousarray(smalls_r.astype(f)),
        })
    return in_maps


def run_on_device(inputs, trace=False, tmpdir=None):
    if "nc" not in _CACHE:
        _CACHE["nc"] = build_nc()
    nc = _CACHE["nc"]
    in_maps = _prep_inputs(inputs)
    res = run_bass_kernel_spmd(nc, in_maps, list(range(NC)),
                               trace=trace, tmpdir=tmpdir)
    shards = [np.asarray(res.results[r]["y"]) for r in range(NC)]
    yt = np.concatenate(shards, axis=0)                  # [D, BT]
    out = np.ascontiguousarray(yt.T).reshape(B, T, D).astype(np.float32)
    return out, res


def kernel(**inputs):
    out, _ = run_on_device(inputs, trace=False)
    return out



# revision 14
# speedup vs baseline: 1.0021x; 1.0021x over previous
"""Trainium2 Bass kernel for a Griffin-style ChimeraBlock:
   pre-norm RG-LRU recurrence branch + pre-norm SwiGLU FFN, B=2, T=2048,
   D=H=2048, FFN=5632, fp32 I/O.

Parallelization over 8 NeuronCores (tensor-parallel), v2 (pipelined):
  - recurrence hidden dim H sharded 8x (256/core); native DVE
    tensor_tensor_scan; FFN hidden sharded 8x (704 -> padded 768).
  - rmsnorm scale factors are applied AFTER the matmuls (per-token column
    scale), so matmuls never wait on the stats AllReduces.
  - hs sum-of-squares partials ride the hs AllGather as a 257th row
    (no separate AllReduce); xnew stats use small per-pair AllReduces.
  - the whole back half (rec_out -> xnew -> h2 -> AG -> FFN -> RS -> y)
    is pipelined per 2-chunk pair so PE never drains between phases.
  - xnew kept in SBUF (bf16); final y = xnew + ffn_red in f32.
Matmuls run in bf16 (fp32 accumulation in PSUM); scan state and the
a_t decay factors stay fp32 (bf16 a_t would drift on long-horizon
channels); most other element-wise tiles are bf16 for SBUF headroom.
"""

import sys

sys.path.insert(0, "/opt/trn_rl_repo")

import numpy as np
import ml_dtypes

import concourse.bass as bass
import concourse.mybir as mybir
import concourse.tile as tile
from concourse import bacc
from concourse.bass_utils import run_bass_kernel_spmd

BF16 = mybir.dt.bfloat16
F32 = mybir.dt.float32
AF = mybir.ActivationFunctionType
OP = mybir.AluOpType

B, T, D = 2, 2048, 2048
H, FFN = 2048, 5632
NC = 8
HS = H // NC          # 256 hidden shard
DS = D // NC          # 256 d-model shard (output sharding)
FS = FFN // NC        # 704 ffn shard
FSP = 768             # ffn shard padded to a multiple of 128 (pad weights = 0)
BT = B * T            # 4096
CH = 512              # time-chunk (columns)
NCH = BT // CH        # 8 chunks
CPB = T // CH         # 4 chunks per batch element (scan resets at b boundary)
KD = D // 128         # 16 k-tiles when contracting over D
KH = H // 128         # 16 k-tiles when contracting over H
KF = FSP // 128       # 6 k-tiles when contracting over ffn shard
NP = NCH // 2         # 4 chunk-pairs
EPS = 1e-6
CCONST = 8.0

NP_BF16 = ml_dtypes.bfloat16


def _r128(ap):
    # [R, N] dram view -> [128, R//128, N] (partition, k-tile, col)
    return ap.rearrange("(k p) n -> p k n", p=128)


def build_nc(phases=7):
    nc = bacc.Bacc("TRN2", target_bir_lowering=False, debug=False, num_devices=NC)
    rg = [list(range(NC))]

    # ---------------- kernel I/O (per core) ----------------
    xt = nc.dram_tensor("xt", [D, BT], BF16, kind="ExternalInput")      # x^T replicated
    xbf = nc.dram_tensor("xbf", [DS, BT], BF16, kind="ExternalInput")   # bf16 x^T d-shard
    w3 = nc.dram_tensor("w3", [D, 3 * HS], BF16, kind="ExternalInput")  # in|ig|rg lhsT shard
    wro = nc.dram_tensor("wro", [H, DS], BF16, kind="ExternalInput")    # rec_out lhsT d-shard
    wg = nc.dram_tensor("wg", [D, FSP], BF16, kind="ExternalInput")
    wu = nc.dram_tensor("wu", [D, FSP], BF16, kind="ExternalInput")
    wd = nc.dram_tensor("wd", [FSP, D], BF16, kind="ExternalInput")
    # cols: 0 = rec_lambda, 1 = ig bias, 2 = rg bias, 3 = h0
    smalls = nc.dram_tensor("smalls", [HS, 4], F32, kind="ExternalInput")
    y = nc.dram_tensor("y", [DS, BT], F32, kind="ExternalOutput")

    with tile.TileContext(nc) as tc:
        with (
            tc.tile_pool(name="sb", bufs=2) as sb,
            tc.tile_pool(name="ps", bufs=2, space="PSUM") as ps,
            tc.tile_pool(name="dr", bufs=1, space="DRAM") as dr,
        ):
            build_body(nc, tc, sb, ps, dr, rg,
                       xt, xbf, w3, wro, wg, wu, wd, smalls, y, phases)
    nc.compile()
    return nc


def build_body(nc, tc, sb, ps, dr, rg, xt, xbf, w3, wro, wg, wu, wd, smalls, y,
               phases=7):
    def finish_early():
        fin = sb.tile([128, 4], F32, name="fin", tag="fin", bufs=1)
        nc.vector.memset(fin[:], 0.0)
        nc.sync.dma_start(out=y[0:128, 0:4], in_=fin[:])
    AG = "AllGather"
    AR = "AllReduce"
    RS = "ReduceScatter"

    dma = nc.sync.dma_start

    # ---------------- internal DRAM ----------------
    warm_in = dr.tile([1, 8], F32, name="warm_in")
    warm_out = dr.tile([1, 8], F32, name="warm_out", addr_space="Shared")
    ar1_in = dr.tile([1, BT], BF16, name="ar1_in")
    ar1_out = dr.tile([1, BT], BF16, name="ar1_out", addr_space="Shared")
    ar3_in = [dr.tile([1, 2 * CH], BF16, name=f"ar3_in{j}") for j in range(NP)]
    ar3_out = [dr.tile([1, 2 * CH], BF16, name=f"ar3_out{j}", addr_space="Shared")
               for j in range(NP)]
    # hs AG payload: 256 data rows + 1 stats row (bf16 partial sum-of-squares)
    agin_hs = [dr.tile([HS + 1, 2 * CH], BF16, name=f"agin_hs{j}")
               for j in range(NP)]
    agout_hs = [dr.tile([NC * (HS + 1), 2 * CH], BF16, name=f"agout_hs{j}",
                        addr_space="Shared") for j in range(NP)]
    agin_h2 = [dr.tile([DS, 2 * CH], BF16, name=f"agin_h2{j}") for j in range(NP)]
    agout_h2 = [dr.tile([D, 2 * CH], BF16, name=f"agout_h2{j}", addr_space="Shared")
                for j in range(NP)]
    ffn_part = [dr.tile([D, 2 * CH], BF16, name=f"ffn_part{j}") for j in range(NP)]
    ffn_red = [dr.tile([DS, 2 * CH], BF16, name=f"ffn_red{j}") for j in range(NP)]

    # ---------------- warmup collective (absorb first-call cost) ----------------
    wtile = sb.tile([1, 8], F32, name="wtile", tag="wtile", bufs=1)
    nc.vector.memset(wtile[:], 0.0)
    dma(out=warm_in[:], in_=wtile[:])
    nc.gpsimd.collective_compute(AR, OP.add, replica_groups=rg,
                                 ins=[warm_in[:]], outs=[warm_out[:]])

    # ---------------- constants / small tensors ----------------
    ones_bf = sb.tile([128, 1], BF16, name="ones_bf", tag="ones", bufs=1)
    nc.vector.memset(ones_bf[:], 1.0)

    def const_tile(val, cname):
        t = sb.tile([128, 1], F32, name=cname, tag=cname, bufs=1)
        nc.vector.memset(t[:], val)
        return t

    # 32-row ones/zeros column for the K=8 stats reduction: rows 8-31 must be
    # explicit zeros (stale PE rows / SBUF garbage otherwise contaminate K<32
    # contractions, which always run as a 32-row hardware tile)
    ones8z = sb.tile([32, 1], BF16, name="ones8z", tag="ones8z", bufs=1)
    nc.vector.memset(ones8z[:], 0.0)
    nc.vector.memset(ones8z[0:8, :], 1.0)  # base partition 0, size 8

    c_ln8 = const_tile(1e-8, "c_ln8")       # Ln bias
    c_eps = const_tile(EPS, "c_eps")        # rmsnorm eps
    c_1eps = const_tile(1.0 + EPS, "c_1eps")  # 1 + eps for sqrt(1 - a^2 + eps)

    smalls_sb = sb.tile([128, 2, 4], F32, name="smalls_sb", tag="smalls", bufs=1)
    dma(out=smalls_sb[:], in_=smalls[:].rearrange("(a p) c -> p a c", p=128))
    sig_l = sb.tile([128, 2], F32, name="sig_l", tag="sig_l", bufs=1)
    nc.scalar.activation(sig_l[:], smalls_sb[:, :, 0], AF.Sigmoid)
    c8_sb = sb.tile([128, 2], F32, name="c8_sb", tag="c8", bufs=1)
    # log(sigmoid(lambda) + 1e-8)
    nc.scalar.activation(c8_sb[:], sig_l[:], AF.Ln, bias=c_ln8[:])
    # * C (in place via Copy with scale)
    nc.scalar.activation(c8_sb[:], c8_sb[:], AF.Copy, bias=0.0, scale=CCONST)

    # ---------------- weights ----------------
    # bigw slots rotate: w3, wg, wu early; wd reuses w3's slot after in-proj.
    w3_sb = sb.tile([128, KD, 3 * HS], BF16, name="w3_sb", tag="bigw", bufs=3)
    for q in range(4):
        dma(out=w3_sb[:, 4 * q:4 * (q + 1), :],
            in_=_r128(w3[:])[:, 4 * q:4 * (q + 1), :])
    wro_sb = sb.tile([128, KH, DS], BF16, name="wro_sb", tag="wro", bufs=1)
    dma(out=wro_sb[:], in_=_r128(wro[:]))
    wg_sb = sb.tile([128, KD, FSP], BF16, name="wg_sb", tag="bigw", bufs=3)
    dma(out=wg_sb[:], in_=_r128(wg[:]))
    wu_sb = sb.tile([128, KD, FSP], BF16, name="wu_sb", tag="bigw", bufs=3)
    dma(out=wu_sb[:], in_=_r128(wu[:]))

    # ---------------- phase 1: norm1 stats (partial over d-shard) ----------------
    for c in range(NCH):
        cs = slice(c * CH, (c + 1) * CH)
        xft = sb.tile([128, 2, CH], BF16, name=f"xft{c}", tag="xf", bufs=2)
        dma(out=xft[:], in_=_r128(xbf[:])[:, :, cs])
        xsq = sb.tile([128, 2, CH], BF16, name=f"xsq{c}", tag="sq", bufs=2)
        nc.scalar.activation(xsq[:], xft[:], AF.Square)
        psq = ps.tile([1, CH], F32, name=f"psq1_{c}", tag="psq", bufs=2)
        nc.tensor.matmul(psq[:], ones_bf[:], xsq[:, 0, :], start=True, stop=False)
        nc.tensor.matmul(psq[:], ones_bf[:], xsq[:, 1, :], start=False, stop=True)
        sqs = sb.tile([1, CH], BF16, name=f"sqs1_{c}", tag="sqs", bufs=2)
        nc.scalar.copy(sqs[:], psq[:])
        dma(out=ar1_in[0:1, cs], in_=sqs[:])
    nc.gpsimd.collective_compute(AR, OP.add, replica_groups=rg,
                                 ins=[ar1_in[:]], outs=[ar1_out[:]])

    if phases < 2:
        finish_early()
        return

    # ---------------- phase 2: in-proj + gates + scan ----------------
    hst_prev = None
    for c in range(NCH):
        cs = slice(c * CH, (c + 1) * CH)
        j, jj = c // 2, c % 2

        xc = sb.tile([128, KD, CH], BF16, name=f"xc{c}", tag="stream", bufs=2)
        dma(out=xc[:], in_=_r128(xt[:])[:, :, cs])

        # matmuls first: they do not depend on AR1
        pst = {}
        for g in range(6):  # (proj, m): 0,1 x_proj; 2,3 ig; 4,5 rg
            p = ps.tile([128, CH], F32, name=f"pp{c}_{g}", tag="mm", bufs=6)
            for k in range(KD):
                nc.tensor.matmul(
                    p[:], w3_sb[:, k, g * 128:(g + 1) * 128], xc[:, k, :],
                    start=(k == 0), stop=(k == KD - 1))
            pst[g] = p

        # inv_rms1 for this chunk (scale applied after the matmul)
        arc = sb.tile([1, CH], BF16, name=f"arc{c}", tag="arc", bufs=2)
        dma(out=arc[:], in_=ar1_out[0:1, cs])
        rsq = sb.tile([1, CH], F32, name=f"rsq1_{c}", tag="srow", bufs=2)
        nc.scalar.activation(rsq[:], arc[:], AF.Sqrt,
                             bias=c_eps[:1, :], scale=1.0 / D)
        rsqb = sb.tile([1, CH], BF16, name=f"rsqb1_{c}", tag="rsqb", bufs=2)
        with nc.allow_low_precision(reason="bf16 per-token inv_rms scale"):
            nc.vector.reciprocal(rsqb[:], rsq[:])
        invc = sb.tile([128, CH], BF16, name=f"invc1_{c}", tag="invc", bufs=4)
        nc.gpsimd.partition_broadcast(invc[:], rsqb[:])

        zt = sb.tile([128, 6, CH], BF16, name=f"z{c}", tag="zgu", bufs=2)
        for g in range(6):
            nc.vector.tensor_tensor(zt[:, g, :], pst[g][:], invc[:], op=OP.mult)

        hst = sb.tile([128, 2, CH], BF16, name=f"hst{c}", tag="hs", bufs=3)
        # ACT ops grouped by function to limit activation-table swaps
        it = {}
        rt = {}
        for m in range(2):
            it[m] = sb.tile([128, CH], BF16, name=f"it{c}_{m}", tag=f"it{m}", bufs=2)
            nc.scalar.activation(it[m][:], zt[:, 2 + m, :], AF.Sigmoid,
                                 bias=smalls_sb[:, m, 1:2])
            rt[m] = sb.tile([128, CH], BF16, name=f"rt{c}_{m}", tag=f"rt{m}", bufs=2)
            nc.scalar.activation(rt[m][:], zt[:, 4 + m, :], AF.Sigmoid,
                                 bias=smalls_sb[:, m, 2:3])
        at = sb.tile([128, 2, CH], F32, name=f"at{c}", tag="at", bufs=1)
        for m in range(2):
            # la = r * (C * log_a)  (in place over rt)
            nc.vector.tensor_scalar_mul(rt[m][:], rt[m][:], c8_sb[:, m:m + 1])
            nc.scalar.activation(at[:, m, :], rt[m][:], AF.Exp)
        na = {}
        for m in range(2):
            # f32: 1 - a^2 suffers catastrophic bf16 quantization near a = 1
            na[m] = sb.tile([128, CH], F32, name=f"na{c}_{m}", tag=f"na{m}", bufs=2)
            nc.vector.scalar_tensor_tensor(na[m][:], at[:, m, :], -1.0, at[:, m, :],
                                           op0=OP.mult, op1=OP.mult)
        for m in range(2):
            nc.scalar.activation(na[m][:], na[m][:], AF.Sqrt, bias=c_1eps[:])
        for m in range(2):
            # g = sq * (i * x_proj)   (in place over the x_proj z-slice)
            nc.vector.tensor_tensor(zt[:, m, :], it[m][:], zt[:, m, :], op=OP.mult)
            nc.vector.tensor_tensor(zt[:, m, :], na[m][:], zt[:, m, :], op=OP.mult)
            if c % CPB == 0:
                init = smalls_sb[:, m, 3:4]
            else:
                init = hst_prev[:, m, CH - 1:CH]
            nc.vector.tensor_tensor_scan(hst[:, m, :], at[:, m, :], zt[:, m, :],
                                         init, op0=OP.mult, op1=OP.add)
        hst_prev = hst

        # partial sumsq of hs over the h-shard -> stats row of the AG payload
        hsq = sb.tile([128, 2, CH], BF16, name=f"hsq{c}", tag="sq", bufs=2)
        nc.scalar.activation(hsq[:], hst[:], AF.Square)
        psq2 = ps.tile([1, CH], F32, name=f"psq2_{c}", tag="psq", bufs=2)
        nc.tensor.matmul(psq2[:], ones_bf[:], hsq[:, 0, :], start=True, stop=False)
        nc.tensor.matmul(psq2[:], ones_bf[:], hsq[:, 1, :], start=False, stop=True)
        sqs2 = sb.tile([1, CH], BF16, name=f"sqs2_{c}", tag="sqs", bufs=2)
        nc.scalar.copy(sqs2[:], psq2[:])
        jsl = slice(jj * CH, (jj + 1) * CH)
        dma(out=agin_hs[j][0:HS, jsl].rearrange("(a p) n -> p a n", p=128),
            in_=hst[:])
        dma(out=agin_hs[j][HS:HS + 1, jsl], in_=sqs2[:])
        if jj == 1:
            nc.gpsimd.collective_compute(AG, OP.bypass, replica_groups=rg,
                                         ins=[agin_hs[j][:]], outs=[agout_hs[j][:]])

    if phases < 4:
        finish_early()
        return

    # ------- merged back half: per pair j, rec_out/residual/h2/AG for pair j
    # ------- then FFN + RS + final residual for pair j-1 (1-pair SW pipeline)
    wd_sb = sb.tile([128, KF, D], BF16, name="wd_sb", tag="bigw", bufs=3)
    dma(out=wd_sb[:], in_=_r128(wd[:]))

    xnt_t = {}

    def phase4_chunk(c, invc2_pair):
        cs = slice(c * CH, (c + 1) * CH)
        j, jj = c // 2, c % 2
        # agout_hs rows: q*(HS+1) + [0, 256) data, q*(HS+1)+256 stats
        hs_view = agout_hs[j][:].rearrange("(q x) n -> q x n", x=HS + 1)

        if jj == 0:
            # per-pair: sum the 8 bf16 stats partials -> invc2 over both chunks
            st2 = sb.tile([32, 1, 2 * CH], BF16, name=f"st2_{j}", tag="st2", bufs=1)
            nc.vector.memset(st2[:], 0.0)
            dma(out=st2[0:8, :, :], in_=hs_view[:, HS:HS + 1, :])
            rsq2 = sb.tile([1, 2 * CH], F32, name=f"rsq2_{j}", tag="srow", bufs=2)
            for half in range(2):
                hsl = slice(half * CH, (half + 1) * CH)
                pst2 = ps.tile([1, CH], F32, name=f"ps2_{j}_{half}", tag="psq",
                               bufs=2)
                nc.tensor.matmul(pst2[:], ones8z[:], st2[:, 0, hsl],
                                 start=True, stop=True)
                nc.scalar.activation(rsq2[:, hsl], pst2[:], AF.Sqrt,
                                     bias=c_eps[:1, :], scale=1.0 / H)
            rsqb2 = sb.tile([1, 2 * CH], BF16, name=f"rsqb2_{j}", tag="rsqb2",
                            bufs=1)
            with nc.allow_low_precision(reason="bf16 per-token inv_rms scale"):
                nc.vector.reciprocal(rsqb2[:], rsq2[:])
            inv2 = {}
            for jj2 in range(2):
                inv2[jj2] = sb.tile([128, CH], BF16, name=f"invc2_{j}_{jj2}",
                                    tag="invc", bufs=4)
                nc.gpsimd.partition_broadcast(
                    inv2[jj2][:], rsqb2[0:1, jj2 * CH:(jj2 + 1) * CH])
            invc2_pair = inv2

        hstm = sb.tile([128, KH // 2, 2, CH], BF16, name=f"hstm{c}", tag="stream",
                       bufs=2)
        for h in range(2):
            dma(out=hstm[:, :, h, :],
                in_=hs_view[:, 0:HS, :]
                .rearrange("q (h p) n -> p q h n", p=128)
                [:, :, h, jj * CH:(jj + 1) * CH])
        xft2 = sb.tile([128, 2, CH], BF16, name=f"xfr{c}", tag="xf", bufs=2)
        dma(out=xft2[:], in_=_r128(xbf[:])[:, :, cs])

        xnt = sb.tile([128, 2, CH], BF16, name=f"xnt{c}", tag="xnt", bufs=5)
        xnt_t[c] = xnt
        for m in range(2):
            pro = ps.tile([128, CH], F32, name=f"pro{c}_{m}", tag="mm", bufs=6)
            for kt in range(KH):
                nc.tensor.matmul(pro[:], wro_sb[:, kt, m * 128:(m + 1) * 128],
                                 hstm[:, kt // 2, kt % 2, :],
                                 start=(kt == 0), stop=(kt == KH - 1))
            nc.vector.tensor_tensor(xnt[:, m, :], pro[:], invc2_pair[jj][:],
                                    op=OP.mult)
            nc.vector.tensor_tensor(xnt[:, m, :], xnt[:, m, :], xft2[:, m, :],
                                    op=OP.add)

        # xnew stats for this chunk -> per-pair AllReduce
        xnq = sb.tile([128, 2, CH], BF16, name=f"xnq{c}", tag="sq", bufs=2)
        nc.scalar.activation(xnq[:], xnt[:], AF.Square)
        psq3 = ps.tile([1, CH], F32, name=f"psq3_{c}", tag="psq", bufs=2)
        nc.tensor.matmul(psq3[:], ones_bf[:], xnq[:, 0, :], start=True, stop=False)
        nc.tensor.matmul(psq3[:], ones_bf[:], xnq[:, 1, :], start=False, stop=True)
        sqs3 = sb.tile([1, CH], BF16, name=f"sqs3_{c}", tag="sqs", bufs=2)
        nc.scalar.copy(sqs3[:], psq3[:])
        dma(out=ar3_in[j][0:1, jj * CH:(jj + 1) * CH], in_=sqs3[:])

        if jj == 1:
            nc.gpsimd.collective_compute(AR, OP.add, replica_groups=rg,
                                         ins=[ar3_in[j][:]], outs=[ar3_out[j][:]])
            # h2 = xnew * invc3 for both chunks of the pair, then AG
            arc3 = sb.tile([1, 2 * CH], BF16, name=f"arc3_{j}", tag="arc3", bufs=1)
            nc.gpsimd.dma_start(out=arc3[:], in_=ar3_out[j][:])
            rsq3 = sb.tile([1, 2 * CH], F32, name=f"rsq3_{j}", tag="srow", bufs=2)
            nc.scalar.activation(rsq3[:], arc3[:], AF.Sqrt,
                                 bias=c_eps[:1, :], scale=1.0 / D)
            rsqb3 = sb.tile([1, 2 * CH], BF16, name=f"rsqb3_{j}", tag="rsqb2",
                            bufs=1)
            with nc.allow_low_precision(reason="bf16 per-token inv_rms scale"):
                nc.vector.reciprocal(rsqb3[:], rsq3[:])
            for cc in (c - 1, c):
                ccj = cc % 2
                inv3 = sb.tile([128, CH], BF16, name=f"invc3_{j}_{ccj}",
                               tag="invc", bufs=4)
                nc.gpsimd.partition_broadcast(
                    inv3[:], rsqb3[0:1, ccj * CH:(ccj + 1) * CH])
                h2t = sb.tile([128, 2, CH], BF16, name=f"h2t{j}_{ccj}", tag="h2t",
                              bufs=2)
                for m in range(2):
                    nc.vector.tensor_tensor(h2t[:, m, :], xnt_t[cc][:, m, :],
                                            inv3[:], op=OP.mult)
                dma(out=agin_h2[j][:, ccj * CH:(ccj + 1) * CH]
                    .rearrange("(a p) n -> p a n", p=128), in_=h2t[:])
            nc.gpsimd.collective_compute(AG, OP.bypass, replica_groups=rg,
                                         ins=[agin_h2[j][:]], outs=[agout_h2[j][:]])
        return invc2_pair

    def phase6_chunk(c):
        j, jj = c // 2, c % 2
        h2s = sb.tile([128, KD, CH], BF16, name=f"h2s{c}", tag="stream", bufs=2)
        dma(out=h2s[:], in_=_r128(agout_h2[j][:])[:, :, jj * CH:(jj + 1) * CH])
        gu = sb.tile([128, KF, CH], BF16, name=f"gu{c}", tag="zgu", bufs=2)
        for m in range(KF):
            psg = ps.tile([128, CH], F32, name=f"pg{c}_{m}", tag="mm", bufs=6)
            for k in range(KD):
                nc.tensor.matmul(psg[:], wg_sb[:, k, m * 128:(m + 1) * 128],
                                 h2s[:, k, :],
                                 start=(k == 0), stop=(k == KD - 1))
            gs = sb.tile([128, CH], BF16, name=f"gs{c}_{m}", tag="gsil", bufs=2)
            nc.scalar.activation(gs[:], psg[:], AF.Silu)
            psu = ps.tile([128, CH], F32, name=f"pu{c}_{m}", tag="mm", bufs=6)
            for k in range(KD):
                nc.tensor.matmul(psu[:], wu_sb[:, k, m * 128:(m + 1) * 128],
                                 h2s[:, k, :],
                                 start=(k == 0), stop=(k == KD - 1))
            nc.vector.tensor_tensor(gu[:, m, :], psu[:], gs[:], op=OP.mult)
        for m in range(KD):
            psd = ps.tile([128, CH], F32, name=f"pd{c}_{m}", tag="mm", bufs=6)
            for k in range(KF):
                nc.tensor.matmul(psd[:], wd_sb[:, k, m * 128:(m + 1) * 128],
                                 gu[:, k, :],
                                 start=(k == 0), stop=(k == KF - 1))
            dst = sb.tile([128, CH], BF16, name=f"dst{c}_{m}", tag="dstage", bufs=2)
            if m % 2 == 0:
                nc.vector.tensor_copy(dst[:], psd[:])
            else:
                nc.scalar.copy(dst[:], psd[:])
            dma(out=ffn_part[j][m * 128:(m + 1) * 128, jj * CH:(jj + 1) * CH],
                in_=dst[:])
        if jj == 1:
            nc.gpsimd.collective_compute(RS, OP.add, replica_groups=rg,
                                         ins=[ffn_part[j][:]], outs=[ffn_red[j][:]])

    def phase7_chunk(c):
        cs = slice(c * CH, (c + 1) * CH)
        j, jj = c // 2, c % 2
        for m in range(2):
            frt = sb.tile([128, CH], BF16, name=f"frt{c}_{m}", tag="frt", bufs=2)
            dma(out=frt[:],
                in_=_r128(ffn_red[j][:])[:, m, jj * CH:(jj + 1) * CH])
            yt = sb.tile([128, CH], F32, name=f"yt{c}_{m}", tag="yt", bufs=2)
            nc.vector.tensor_tensor(yt[:], xnt_t[c][:, m, :], frt[:], op=OP.add)
            dma(out=_r128(y[:])[:, m, cs], in_=yt[:])

    run_ffn = phases >= 6
    for j in range(NP + 1):
        invc2_pair = None
        if j < NP:
            invc2_pair = phase4_chunk(2 * j, None)
            invc2_pair = phase4_chunk(2 * j + 1, invc2_pair)
        if run_ffn and j >= 1:
            jf = j - 1
            phase6_chunk(2 * jf)
            phase6_chunk(2 * jf + 1)
            if phases >= 7:
                phase7_chunk(2 * jf)
                phase7_chunk(2 * jf + 1)

    if phases < 6:
        # debug: dump xnew to y
        for c in range(NCH):
            cs = slice(c * CH, (c + 1) * CH)
            ytd = sb.tile([128, 2, CH], F32, name=f"ytd{c}", tag="yt", bufs=2)
            nc.vector.tensor_copy(ytd[:], xnt_t[c][:])
            dma(out=_r128(y[:])[:, :, cs], in_=ytd[:])
        return
    if phases < 7:
        finish_early()
        return


_CACHE = {}


def _prep_inputs(inputs):
    f = np.float32
    x = np.asarray(inputs["x"], f)                       # [B, T, D]
    norm1_w = np.asarray(inputs["norm1_w"], f)
    rec_in_w = np.asarray(inputs["rec_in_w"], f)         # [H, D]
    rec_ig_w = np.asarray(inputs["rec_ig_w"], f)
    rec_ig_b = np.asarray(inputs["rec_ig_b"], f)
    rec_rg_w = np.asarray(inputs["rec_rg_w"], f)
    rec_rg_b = np.asarray(inputs["rec_rg_b"], f)
    rec_lambda = np.asarray(inputs["rec_lambda"], f)
    rec_out_w = np.asarray(inputs["rec_out_w"], f)       # [D, H]
    rec_h0 = np.asarray(inputs["rec_h0"], f)             # [1, 1, H]
    rec_norm_w = np.asarray(inputs["rec_norm_w"], f)
    norm2_w = np.asarray(inputs["norm2_w"], f)
    ffn_gate_w = np.asarray(inputs["ffn_gate_w"], f)     # [FFN, D]
    ffn_up_w = np.asarray(inputs["ffn_up_w"], f)
    ffn_down_w = np.asarray(inputs["ffn_down_w"], f)     # [D, FFN]

    xt_full = np.ascontiguousarray(
        x.reshape(BT, D).T.astype(NP_BF16))              # [D, BT]

    # fold norm gains into adjacent weights; transpose into lhsT layouts
    w_in_t = (rec_in_w * norm1_w[None, :]).T             # [D, H]
    w_ig_t = (rec_ig_w * norm1_w[None, :]).T
    w_rg_t = (rec_rg_w * norm1_w[None, :]).T
    w_ro_t = (rec_out_w * rec_norm_w[None, :]).T         # [H, D]
    w_g_t = (ffn_gate_w * norm2_w[None, :]).T            # [D, FFN]
    w_u_t = (ffn_up_w * norm2_w[None, :]).T
    w_d_t = ffn_down_w.T                                 # [FFN, D]

    in_maps = []
    for r in range(NC):
        hsl = slice(r * HS, (r + 1) * HS)
        dsl = slice(r * DS, (r + 1) * DS)
        fsl = slice(r * FS, (r + 1) * FS)
        w3_r = np.concatenate(
            [w_in_t[:, hsl], w_ig_t[:, hsl], w_rg_t[:, hsl]], axis=1)
        wg_r = np.zeros((D, FSP), f)
        wg_r[:, :FS] = w_g_t[:, fsl]
        wu_r = np.zeros((D, FSP), f)
        wu_r[:, :FS] = w_u_t[:, fsl]
        wd_r = np.zeros((FSP, D), f)
        wd_r[:FS, :] = w_d_t[fsl, :]
        smalls_r = np.stack(
            [rec_lambda[hsl], rec_ig_b[hsl], rec_rg_b[hsl],
             np.broadcast_to(rec_h0[0, 0], (H,))[hsl]], axis=1)
        in_maps.append({
            "xt": xt_full,
            "xbf": np.ascontiguousarray(xt_full[dsl, :]),
            "w3": np.ascontiguousarray(w3_r.astype(NP_BF16)),
            "wro": np.ascontiguousarray(w_ro_t[:, dsl].astype(NP_BF16)),
            "wg": np.ascontiguousarray(wg_r.astype(NP_BF16)),
            "wu": np.ascontiguousarray(wu_r.astype(NP_BF16)),
            "wd": np.ascontiguousarray(wd_r.astype(NP_BF16)),
            "smalls": np.ascontiguousarray(smalls_r.astype(f)),
        })
    return in_maps


def run_on_device(inputs, trace=False, tmpdir=None):
    if "nc" not in _CACHE:
        _CACHE["nc"] = build_nc()
    nc = _CACHE["nc"]
    in_maps = _prep_inputs(inputs)
    res = run_bass_kernel_spmd(nc, in_maps, list(range(NC)),
                               trace=trace, tmpdir=tmpdir)
    shards = [np.asarray(res.results[r]["y"]) for r in range(NC)]
    yt = np.concatenate(shards, axis=0)                  # [D, BT]
    out = np.ascontiguousarray(yt.T).reshape(B, T, D).astype(np.float32)
    return out, res


def kernel(**inputs):
    out, _ = run_on_device(inputs, trace=False)
    return out


# revision 15
# speedup vs baseline: 1.0117x; 1.0096x over previous
"""Trainium2 Bass kernel for a Griffin-style ChimeraBlock:
   pre-norm RG-LRU recurrence branch + pre-norm SwiGLU FFN, B=2, T=2048,
   D=H=2048, FFN=5632, fp32 I/O.

Parallelization over 8 NeuronCores (tensor-parallel), v2 (pipelined):
  - recurrence hidden dim H sharded 8x (256/core); native DVE
    tensor_tensor_scan; FFN hidden sharded 8x (704 -> padded 768).
  - rmsnorm scale factors are applied AFTER the matmuls (per-token column
    scale), so matmuls never wait on the stats AllReduces.
  - hs sum-of-squares partials ride the hs AllGather as a 257th row
    (no separate AllReduce); xnew stats use small per-pair AllReduces.
  - the whole back half (rec_out -> xnew -> h2 -> AG -> FFN -> RS -> y)
    is pipelined per 2-chunk pair so PE never drains between phases.
  - xnew kept in SBUF (bf16); final y = xnew + ffn_red in f32.
Matmuls run in bf16 (fp32 accumulation in PSUM); scan state and the
a_t decay factors stay fp32 (bf16 a_t would drift on long-horizon
channels); most other element-wise tiles are bf16 for SBUF headroom.
"""

import sys

sys.path.insert(0, "/opt/trn_rl_repo")

import numpy as np
import ml_dtypes

import concourse.bass as bass
import concourse.mybir as mybir
import concourse.tile as tile
from concourse import bacc
from concourse.bass_utils import run_bass_kernel_spmd

BF16 = mybir.dt.bfloat16
F32 = mybir.dt.float32
AF = mybir.ActivationFunctionType
OP = mybir.AluOpType

B, T, D = 2, 2048, 2048
H, FFN = 2048, 5632
NC = 8
HS = H // NC          # 256 hidden shard
DS = D // NC          # 256 d-model shard (output sharding)
FS = FFN // NC        # 704 ffn shard
FSP = 768             # ffn shard padded to a multiple of 128 (pad weights = 0)
BT = B * T            # 4096
CH = 512              # time-chunk (columns)
NCH = BT // CH        # 8 chunks
CPB = T // CH         # 4 chunks per batch element (scan resets at b boundary)
KD = D // 128         # 16 k-tiles when contracting over D
KH = H // 128         # 16 k-tiles when contracting over H
KF = FSP // 128       # 6 k-tiles when contracting over ffn shard
NP = NCH // 2         # 4 chunk-pairs
EPS = 1e-6
CCONST = 8.0

NP_BF16 = ml_dtypes.bfloat16


def _r128(ap):
    # [R, N] dram view -> [128, R//128, N] (partition, k-tile, col)
    return ap.rearrange("(k p) n -> p k n", p=128)


def build_nc(phases=7):
    nc = bacc.Bacc("TRN2", target_bir_lowering=False, debug=False, num_devices=NC)
    rg = [list(range(NC))]

    # ---------------- kernel I/O (per core) ----------------
    xt = nc.dram_tensor("xt", [D, BT], BF16, kind="ExternalInput")      # x^T replicated
    xbf = nc.dram_tensor("xbf", [DS, BT], BF16, kind="ExternalInput")   # bf16 x^T d-shard
    w3 = nc.dram_tensor("w3", [D, 3 * HS], BF16, kind="ExternalInput")  # in|ig|rg lhsT shard
    wro = nc.dram_tensor("wro", [H, DS], BF16, kind="ExternalInput")    # rec_out lhsT d-shard
    wg = nc.dram_tensor("wg", [D, FSP], BF16, kind="ExternalInput")
    wu = nc.dram_tensor("wu", [D, FSP], BF16, kind="ExternalInput")
    wd = nc.dram_tensor("wd", [FSP, D], BF16, kind="ExternalInput")
    # cols: 0 = rec_lambda, 1 = ig bias, 2 = rg bias, 3 = h0
    smalls = nc.dram_tensor("smalls", [HS, 4], F32, kind="ExternalInput")
    y = nc.dram_tensor("y", [DS, BT], F32, kind="ExternalOutput")

    with tile.TileContext(nc) as tc:
        with (
            tc.tile_pool(name="sb", bufs=2) as sb,
            tc.tile_pool(name="ps", bufs=2, space="PSUM") as ps,
            tc.tile_pool(name="dr", bufs=1, space="DRAM") as dr,
        ):
            build_body(nc, tc, sb, ps, dr, rg,
                       xt, xbf, w3, wro, wg, wu, wd, smalls, y, phases)
    nc.compile()
    return nc


def build_body(nc, tc, sb, ps, dr, rg, xt, xbf, w3, wro, wg, wu, wd, smalls, y,
               phases=7):
    def finish_early():
        fin = sb.tile([128, 4], F32, name="fin", tag="fin", bufs=1)
        nc.vector.memset(fin[:], 0.0)
        nc.sync.dma_start(out=y[0:128, 0:4], in_=fin[:])
    AG = "AllGather"
    AR = "AllReduce"
    RS = "ReduceScatter"

    dma = nc.sync.dma_start

    # ---------------- internal DRAM ----------------
    warm_in = dr.tile([1, 8], F32, name="warm_in")
    warm_out = dr.tile([1, 8], F32, name="warm_out", addr_space="Shared")
    ar1_in = dr.tile([1, BT], BF16, name="ar1_in")
    ar1_out = dr.tile([1, BT], BF16, name="ar1_out", addr_space="Shared")
    ar3_in = [dr.tile([1, 2 * CH], BF16, name=f"ar3_in{j}") for j in range(NP)]
    ar3_out = [dr.tile([1, 2 * CH], BF16, name=f"ar3_out{j}", addr_space="Shared")
               for j in range(NP)]
    # hs AG payload: 256 data rows + 1 stats row (bf16 partial sum-of-squares)
    agin_hs = [dr.tile([HS + 1, 2 * CH], BF16, name=f"agin_hs{j}")
               for j in range(NP)]
    agout_hs = [dr.tile([NC * (HS + 1), 2 * CH], BF16, name=f"agout_hs{j}",
                        addr_space="Shared") for j in range(NP)]
    agin_h2 = [dr.tile([DS, 2 * CH], BF16, name=f"agin_h2{j}") for j in range(NP)]
    agout_h2 = [dr.tile([D, 2 * CH], BF16, name=f"agout_h2{j}", addr_space="Shared")
                for j in range(NP)]
    ffn_part = [dr.tile([D, 2 * CH], BF16, name=f"ffn_part{j}") for j in range(NP)]
    ffn_red = [dr.tile([DS, 2 * CH], BF16, name=f"ffn_red{j}") for j in range(NP)]

    # ---------------- warmup collective (absorb first-call cost) ----------------
    wtile = sb.tile([1, 8], F32, name="wtile", tag="wtile", bufs=1)
    nc.vector.memset(wtile[:], 0.0)
    dma(out=warm_in[:], in_=wtile[:])
    nc.gpsimd.collective_compute(AR, OP.add, replica_groups=rg,
                                 ins=[warm_in[:]], outs=[warm_out[:]])

    # ---------------- constants / small tensors ----------------
    ones_bf = sb.tile([128, 1], BF16, name="ones_bf", tag="ones", bufs=1)
    nc.vector.memset(ones_bf[:], 1.0)

    def const_tile(val, cname):
        t = sb.tile([128, 1], F32, name=cname, tag=cname, bufs=1)
        nc.vector.memset(t[:], val)
        return t

    # 32-row ones/zeros column for the K=8 stats reduction: rows 8-31 must be
    # explicit zeros (stale PE rows / SBUF garbage otherwise contaminate K<32
    # contractions, which always run as a 32-row hardware tile)
    ones8z = sb.tile([32, 1], BF16, name="ones8z", tag="ones8z", bufs=1)
    nc.vector.memset(ones8z[:], 0.0)
    nc.vector.memset(ones8z[0:8, :], 1.0)  # base partition 0, size 8

    c_ln8 = const_tile(1e-8, "c_ln8")       # Ln bias
    c_eps = const_tile(EPS, "c_eps")        # rmsnorm eps
    c_1eps = const_tile(1.0 + EPS, "c_1eps")  # 1 + eps for sqrt(1 - a^2 + eps)

    smalls_sb = sb.tile([128, 2, 4], F32, name="smalls_sb", tag="smalls", bufs=1)
    dma(out=smalls_sb[:], in_=smalls[:].rearrange("(a p) c -> p a c", p=128))
    sig_l = sb.tile([128, 2], F32, name="sig_l", tag="sig_l", bufs=1)
    nc.scalar.activation(sig_l[:], smalls_sb[:, :, 0], AF.Sigmoid)
    c8_sb = sb.tile([128, 2], F32, name="c8_sb", tag="c8", bufs=1)
    # log(sigmoid(lambda) + 1e-8)
    nc.scalar.activation(c8_sb[:], sig_l[:], AF.Ln, bias=c_ln8[:])
    # * C (in place via Copy with scale)
    nc.scalar.activation(c8_sb[:], c8_sb[:], AF.Copy, bias=0.0, scale=CCONST)

    # ---------------- weights ----------------
    # bigw slots rotate: w3, wg, wu early; wd reuses w3's slot after in-proj.
    w3_sb = sb.tile([128, KD, 3 * HS], BF16, name="w3_sb", tag="bigw", bufs=3)
    for q in range(4):
        dma(out=w3_sb[:, 4 * q:4 * (q + 1), :],
            in_=_r128(w3[:])[:, 4 * q:4 * (q + 1), :])
    # ---------------- phase 1: norm1 stats (partial over d-shard) ----------------
    for c in range(NCH):
        cs = slice(c * CH, (c + 1) * CH)
        xft = sb.tile([128, 2, CH], BF16, name=f"xft{c}", tag="xf", bufs=2)
        dma(out=xft[:], in_=_r128(xbf[:])[:, :, cs])
        xsq = sb.tile([128, 2, CH], BF16, name=f"xsq{c}", tag="sq", bufs=2)
        nc.scalar.activation(xsq[:], xft[:], AF.Square)
        psq = ps.tile([1, CH], F32, name=f"psq1_{c}", tag="psq", bufs=2)
        nc.tensor.matmul(psq[:], ones_bf[:], xsq[:, 0, :], start=True, stop=False)
        nc.tensor.matmul(psq[:], ones_bf[:], xsq[:, 1, :], start=False, stop=True)
        sqs = sb.tile([1, CH], BF16, name=f"sqs1_{c}", tag="sqs", bufs=2)
        nc.scalar.copy(sqs[:], psq[:])
        dma(out=ar1_in[0:1, cs], in_=sqs[:])
    nc.gpsimd.collective_compute(AR, OP.add, replica_groups=rg,
                                 ins=[ar1_in[:]], outs=[ar1_out[:]])

    if phases < 2:
        finish_early()
        return

    # ---------------- phase 2: in-proj + gates + scan ----------------
    late_w = {}

    def load_late_weights():
        late_w["wro"] = sb.tile([128, KH, DS], BF16, name="wro_sb", tag="wro",
                                bufs=1)
        dma(out=late_w["wro"][:], in_=_r128(wro[:]))
        late_w["wg"] = sb.tile([128, KD, FSP], BF16, name="wg_sb", tag="bigw",
                               bufs=3)
        dma(out=late_w["wg"][:], in_=_r128(wg[:]))
        late_w["wu"] = sb.tile([128, KD, FSP], BF16, name="wu_sb", tag="bigw",
                               bufs=3)
        dma(out=late_w["wu"][:], in_=_r128(wu[:]))

    hst_prev = None
    for c in range(NCH):
        cs = slice(c * CH, (c + 1) * CH)
        j, jj = c // 2, c % 2

        xc = sb.tile([128, KD, CH], BF16, name=f"xc{c}", tag="stream", bufs=2)
        dma(out=xc[:], in_=_r128(xt[:])[:, :, cs])

        # matmuls first: they do not depend on AR1
        pst = {}
        for g in range(6):  # (proj, m): 0,1 x_proj; 2,3 ig; 4,5 rg
            p = ps.tile([128, CH], F32, name=f"pp{c}_{g}", tag="mm", bufs=6)
            for k in range(KD):
                nc.tensor.matmul(
                    p[:], w3_sb[:, k, g * 128:(g + 1) * 128], xc[:, k, :],
                    start=(k == 0), stop=(k == KD - 1))
            pst[g] = p

        # raw psum -> SBUF immediately (ACT copy): frees PSUM without AR1
        zt = sb.tile([128, 6, CH], BF16, name=f"z{c}", tag="zgu", bufs=2)
        for g in range(6):
            nc.scalar.copy(zt[:, g, :], pst[g][:])

        # inv_rms1 for this chunk (scale applied after the matmul)
        arc = sb.tile([1, CH], BF16, name=f"arc{c}", tag="arc", bufs=2)
        dma(out=arc[:], in_=ar1_out[0:1, cs])
        rsq = sb.tile([1, CH], F32, name=f"rsq1_{c}", tag="srow", bufs=2)
        nc.scalar.activation(rsq[:], arc[:], AF.Sqrt,
                             bias=c_eps[:1, :], scale=1.0 / D)
        rsqb = sb.tile([1, CH], BF16, name=f"rsqb1_{c}", tag="rsqb", bufs=2)
        with nc.allow_low_precision(reason="bf16 per-token inv_rms scale"):
            nc.vector.reciprocal(rsqb[:], rsq[:])
        invc = sb.tile([128, CH], BF16, name=f"invc1_{c}", tag="invc", bufs=4)
        nc.gpsimd.partition_broadcast(invc[:], rsqb[:])

        for g in range(6):
            nc.vector.tensor_tensor(zt[:, g, :], zt[:, g, :], invc[:], op=OP.mult)

        if c == 1:
            load_late_weights()

        hst = sb.tile([128, 2, CH], BF16, name=f"hst{c}", tag="hs", bufs=3)
        # ACT ops grouped by function to limit activation-table swaps
        it = {}
        rt = {}
        for m in range(2):
            it[m] = sb.tile([128, CH], BF16, name=f"it{c}_{m}", tag=f"it{m}", bufs=2)
            nc.scalar.activation(it[m][:], zt[:, 2 + m, :], AF.Sigmoid,
                                 bias=smalls_sb[:, m, 1:2])
            rt[m] = sb.tile([128, CH], BF16, name=f"rt{c}_{m}", tag=f"rt{m}", bufs=2)
            nc.scalar.activation(rt[m][:], zt[:, 4 + m, :], AF.Sigmoid,
                                 bias=smalls_sb[:, m, 2:3])
        at = sb.tile([128, 2, CH], F32, name=f"at{c}", tag="at", bufs=1)
        for m in range(2):
            # la = r * (C * log_a)  (in place over rt)
            nc.vector.tensor_scalar_mul(rt[m][:], rt[m][:], c8_sb[:, m:m + 1])
            nc.scalar.activation(at[:, m, :], rt[m][:], AF.Exp)
        na = {}
        for m in range(2):
            # f32: 1 - a^2 suffers catastrophic bf16 quantization near a = 1
            na[m] = sb.tile([128, CH], F32, name=f"na{c}_{m}", tag=f"na{m}", bufs=2)
            nc.vector.scalar_tensor_tensor(na[m][:], at[:, m, :], -1.0, at[:, m, :],
                                           op0=OP.mult, op1=OP.mult)
        for m in range(2):
            nc.scalar.activation(na[m][:], na[m][:], AF.Sqrt, bias=c_1eps[:])
        for m in range(2):
            # g = sq * (i * x_proj)   (in place over the x_proj z-slice)
            nc.vector.tensor_tensor(zt[:, m, :], it[m][:], zt[:, m, :], op=OP.mult)
            nc.vector.tensor_tensor(zt[:, m, :], na[m][:], zt[:, m, :], op=OP.mult)
            if c % CPB == 0:
                init = smalls_sb[:, m, 3:4]
            else:
                init = hst_prev[:, m, CH - 1:CH]
            nc.vector.tensor_tensor_scan(hst[:, m, :], at[:, m, :], zt[:, m, :],
                                         init, op0=OP.mult, op1=OP.add)
        hst_prev = hst

        # partial sumsq of hs over the h-shard -> stats row of the AG payload
        hsq = sb.tile([128, 2, CH], BF16, name=f"hsq{c}", tag="sq", bufs=2)
        nc.scalar.activation(hsq[:], hst[:], AF.Square)
        psq2 = ps.tile([1, CH], F32, name=f"psq2_{c}", tag="psq", bufs=2)
        nc.tensor.matmul(psq2[:], ones_bf[:], hsq[:, 0, :], start=True, stop=False)
        nc.tensor.matmul(psq2[:], ones_bf[:], hsq[:, 1, :], start=False, stop=True)
        sqs2 = sb.tile([1, CH], BF16, name=f"sqs2_{c}", tag="sqs", bufs=2)
        nc.scalar.copy(sqs2[:], psq2[:])
        jsl = slice(jj * CH, (jj + 1) * CH)
        dma(out=agin_hs[j][0:HS, jsl].rearrange("(a p) n -> p a n", p=128),
            in_=hst[:])
        dma(out=agin_hs[j][HS:HS + 1, jsl], in_=sqs2[:])
        if jj == 1:
            nc.gpsimd.collective_compute(AG, OP.bypass, replica_groups=rg,
                                         ins=[agin_hs[j][:]], outs=[agout_hs[j][:]])

    if phases < 4:
        finish_early()
        return

    # ------- merged back half: per pair j, rec_out/residual/h2/AG for pair j
    # ------- then FFN + RS + final residual for pair j-1 (1-pair SW pipeline)
    wd_sb = sb.tile([128, KF, D], BF16, name="wd_sb", tag="bigw", bufs=3)
    dma(out=wd_sb[:], in_=_r128(wd[:]))

    xnt_t = {}

    def phase4_chunk(c, invc2_pair):
        cs = slice(c * CH, (c + 1) * CH)
        j, jj = c // 2, c % 2
        # agout_hs rows: q*(HS+1) + [0, 256) data, q*(HS+1)+256 stats
        hs_view = agout_hs[j][:].rearrange("(q x) n -> q x n", x=HS + 1)

        if jj == 0:
            # per-pair: sum the 8 bf16 stats partials -> invc2 over both chunks
            st2 = sb.tile([32, 1, 2 * CH], BF16, name=f"st2_{j}", tag="st2", bufs=1)
            nc.vector.memset(st2[:], 0.0)
            dma(out=st2[0:8, :, :], in_=hs_view[:, HS:HS + 1, :])
            rsq2 = sb.tile([1, 2 * CH], F32, name=f"rsq2_{j}", tag="srow", bufs=2)
            for half in range(2):
                hsl = slice(half * CH, (half + 1) * CH)
                pst2 = ps.tile([1, CH], F32, name=f"ps2_{j}_{half}", tag="psq",
                               bufs=2)
                nc.tensor.matmul(pst2[:], ones8z[:], st2[:, 0, hsl],
                                 start=True, stop=True)
                nc.scalar.activation(rsq2[:, hsl], pst2[:], AF.Sqrt,
                                     bias=c_eps[:1, :], scale=1.0 / H)
            rsqb2 = sb.tile([1, 2 * CH], BF16, name=f"rsqb2_{j}", tag="rsqb2",
                            bufs=1)
            with nc.allow_low_precision(reason="bf16 per-token inv_rms scale"):
                nc.vector.reciprocal(rsqb2[:], rsq2[:])
            inv2 = {}
            for jj2 in range(2):
                inv2[jj2] = sb.tile([128, CH], BF16, name=f"invc2_{j}_{jj2}",
                                    tag="invc", bufs=4)
                nc.gpsimd.partition_broadcast(
                    inv2[jj2][:], rsqb2[0:1, jj2 * CH:(jj2 + 1) * CH])
            invc2_pair = inv2

        hstm = sb.tile([128, KH // 2, 2, CH], BF16, name=f"hstm{c}", tag="stream",
                       bufs=2)
        for h in range(2):
            dma(out=hstm[:, :, h, :],
                in_=hs_view[:, 0:HS, :]
                .rearrange("q (h p) n -> p q h n", p=128)
                [:, :, h, jj * CH:(jj + 1) * CH])
        xft2 = sb.tile([128, 2, CH], BF16, name=f"xfr{c}", tag="xf", bufs=2)
        dma(out=xft2[:], in_=_r128(xbf[:])[:, :, cs])

        xnt = sb.tile([128, 2, CH], BF16, name=f"xnt{c}", tag="xnt", bufs=5)
        xnt_t[c] = xnt
        for m in range(2):
            pro = ps.tile([128, CH], F32, name=f"pro{c}_{m}", tag="mm", bufs=6)
            for kt in range(KH):
                nc.tensor.matmul(pro[:], late_w["wro"][:, kt, m * 128:(m + 1) * 128],
                                 hstm[:, kt // 2, kt % 2, :],
                                 start=(kt == 0), stop=(kt == KH - 1))
            nc.vector.tensor_tensor(xnt[:, m, :], pro[:], invc2_pair[jj][:],
                                    op=OP.mult)
            nc.vector.tensor_tensor(xnt[:, m, :], xnt[:, m, :], xft2[:, m, :],
                                    op=OP.add)

        # xnew stats for this chunk -> per-pair AllReduce
        xnq = sb.tile([128, 2, CH], BF16, name=f"xnq{c}", tag="sq", bufs=2)
        nc.scalar.activation(xnq[:], xnt[:], AF.Square)
        psq3 = ps.tile([1, CH], F32, name=f"psq3_{c}", tag="psq", bufs=2)
        nc.tensor.matmul(psq3[:], ones_bf[:], xnq[:, 0, :], start=True, stop=False)
        nc.tensor.matmul(psq3[:], ones_bf[:], xnq[:, 1, :], start=False, stop=True)
        sqs3 = sb.tile([1, CH], BF16, name=f"sqs3_{c}", tag="sqs", bufs=2)
        nc.scalar.copy(sqs3[:], psq3[:])
        dma(out=ar3_in[j][0:1, jj * CH:(jj + 1) * CH], in_=sqs3[:])

        if jj == 1:
            nc.gpsimd.collective_compute(AR, OP.add, replica_groups=rg,
                                         ins=[ar3_in[j][:]], outs=[ar3_out[j][:]])
            # h2 = xnew * invc3 for both chunks of the pair, then AG
            arc3 = sb.tile([1, 2 * CH], BF16, name=f"arc3_{j}", tag="arc3", bufs=1)
            nc.gpsimd.dma_start(out=arc3[:], in_=ar3_out[j][:])
            rsq3 = sb.tile([1, 2 * CH], F32, name=f"rsq3_{j}", tag="srow", bufs=2)
            nc.scalar.activation(rsq3[:], arc3[:], AF.Sqrt,
                                 bias=c_eps[:1, :], scale=1.0 / D)
            rsqb3 = sb.tile([1, 2 * CH], BF16, name=f"rsqb3_{j}", tag="rsqb2",
                            bufs=1)
            with nc.allow_low_precision(reason="bf16 per-token inv_rms scale"):
                nc.vector.reciprocal(rsqb3[:], rsq3[:])
            for cc in (c - 1, c):
                ccj = cc % 2
                inv3 = sb.tile([128, CH], BF16, name=f"invc3_{j}_{ccj}",
                               tag="invc", bufs=4)
                nc.gpsimd.partition_broadcast(
                    inv3[:], rsqb3[0:1, ccj * CH:(ccj + 1) * CH])
                h2t = sb.tile([128, 2, CH], BF16, name=f"h2t{j}_{ccj}", tag="h2t",
                              bufs=2)
                for m in range(2):
                    nc.vector.tensor_tensor(h2t[:, m, :], xnt_t[cc][:, m, :],
                                            inv3[:], op=OP.mult)
                dma(out=agin_h2[j][:, ccj * CH:(ccj + 1) * CH]
                    .rearrange("(a p) n -> p a n", p=128), in_=h2t[:])
            nc.gpsimd.collective_compute(AG, OP.bypass, replica_groups=rg,
                                         ins=[agin_h2[j][:]], outs=[agout_h2[j][:]])
        return invc2_pair

    def phase6_chunk(c):
        j, jj = c // 2, c % 2
        h2s = sb.tile([128, KD, CH], BF16, name=f"h2s{c}", tag="stream", bufs=2)
        dma(out=h2s[:], in_=_r128(agout_h2[j][:])[:, :, jj * CH:(jj + 1) * CH])
        gu = sb.tile([128, KF, CH], BF16, name=f"gu{c}", tag="zgu", bufs=2)
        for m in range(KF):
            psg = ps.tile([128, CH], F32, name=f"pg{c}_{m}", tag="mm", bufs=6)
            for k in range(KD):
                nc.tensor.matmul(psg[:], late_w["wg"][:, k, m * 128:(m + 1) * 128],
                                 h2s[:, k, :],
                                 start=(k == 0), stop=(k == KD - 1))
            gs = sb.tile([128, CH], BF16, name=f"gs{c}_{m}", tag="gsil", bufs=2)
            nc.scalar.activation(gs[:], psg[:], AF.Silu)
            psu = ps.tile([128, CH], F32, name=f"pu{c}_{m}", tag="mm", bufs=6)
            for k in range(KD):
                nc.tensor.matmul(psu[:], late_w["wu"][:, k, m * 128:(m + 1) * 128],
                                 h2s[:, k, :],
                                 start=(k == 0), stop=(k == KD - 1))
            nc.vector.tensor_tensor(gu[:, m, :], psu[:], gs[:], op=OP.mult)
        for m in range(KD):
            psd = ps.tile([128, CH], F32, name=f"pd{c}_{m}", tag="mm", bufs=6)
            for k in range(KF):
                nc.tensor.matmul(psd[:], wd_sb[:, k, m * 128:(m + 1) * 128],
                                 gu[:, k, :],
                                 start=(k == 0), stop=(k == KF - 1))
            dst = sb.tile([128, CH], BF16, name=f"dst{c}_{m}", tag="dstage", bufs=2)
            if m % 2 == 0:
                nc.vector.tensor_copy(dst[:], psd[:])
            else:
                nc.scalar.copy(dst[:], psd[:])
            dma(out=ffn_part[j][m * 128:(m + 1) * 128, jj * CH:(jj + 1) * CH],
                in_=dst[:])
        if jj == 1:
            nc.gpsimd.collective_compute(RS, OP.add, replica_groups=rg,
                                         ins=[ffn_part[j][:]], outs=[ffn_red[j][:]])

    def phase7_chunk(c):
        cs = slice(c * CH, (c + 1) * CH)
        j, jj = c // 2, c % 2
        for m in range(2):
            frt = sb.tile([128, CH], BF16, name=f"frt{c}_{m}", tag="frt", bufs=2)
            dma(out=frt[:],
                in_=_r128(ffn_red[j][:])[:, m, jj * CH:(jj + 1) * CH])
            yt = sb.tile([128, CH], F32, name=f"yt{c}_{m}", tag="yt", bufs=2)
            nc.vector.tensor_tensor(yt[:], xnt_t[c][:, m, :], frt[:], op=OP.add)
            dma(out=_r128(y[:])[:, m, cs], in_=yt[:])

    run_ffn = phases >= 6
    for j in range(NP + 1):
        invc2_pair = None
        if j < NP:
            invc2_pair = phase4_chunk(2 * j, None)
            invc2_pair = phase4_chunk(2 * j + 1, invc2_pair)
        if run_ffn and j >= 1:
            jf = j - 1
            phase6_chunk(2 * jf)
            phase6_chunk(2 * jf + 1)
            if phases >= 7:
                phase7_chunk(2 * jf)
                phase7_chunk(2 * jf + 1)

    if phases < 6:
        # debug: dump xnew to y
        for c in range(NCH):
            cs = slice(c * CH, (c + 1) * CH)
            ytd = sb.tile([128, 2, CH], F32, name=f"ytd{c}", tag="yt", bufs=2)
            nc.vector.tensor_copy(ytd[:], xnt_t[c][:])
            dma(out=_r128(y[:])[:, :, cs], in_=ytd[:])
        return
    if phases < 7:
        finish_early()
        return


_CACHE = {}


def _prep_inputs(inputs):
    f = np.float32
    x = np.asarray(inputs["x"], f)                       # [B, T, D]
    norm1_w = np.asarray(inputs["norm1_w"], f)
    rec_in_w = np.asarray(inputs["rec_in_w"], f)         # [H, D]
    rec_ig_w = np.asarray(inputs["rec_ig_w"], f)
    rec_ig_b = np.asarray(inputs["rec_ig_b"], f)
    rec_rg_w = np.asarray(inputs["rec_rg_w"], f)
    rec_rg_b = np.asarray(inputs["rec_rg_b"], f)
    rec_lambda = np.asarray(inputs["rec_lambda"], f)
    rec_out_w = np.asarray(inputs["rec_out_w"], f)       # [D, H]
    rec_h0 = np.asarray(inputs["rec_h0"], f)             # [1, 1, H]
    rec_norm_w = np.asarray(inputs["rec_norm_w"], f)
    norm2_w = np.asarray(inputs["norm2_w"], f)
    ffn_gate_w = np.asarray(inputs["ffn_gate_w"], f)     # [FFN, D]
    ffn_up_w = np.asarray(inputs["ffn_up_w"], f)
    ffn_down_w = np.asarray(inputs["ffn_down_w"], f)     # [D, FFN]

    xt_full = np.ascontiguousarray(
        x.reshape(BT, D).T.astype(NP_BF16))              # [D, BT]

    # fold norm gains into adjacent weights; transpose into lhsT layouts
    w_in_t = (rec_in_w * norm1_w[None, :]).T             # [D, H]
    w_ig_t = (rec_ig_w * norm1_w[None, :]).T
    w_rg_t = (rec_rg_w * norm1_w[None, :]).T
    w_ro_t = (rec_out_w * rec_norm_w[None, :]).T         # [H, D]
    w_g_t = (ffn_gate_w * norm2_w[None, :]).T            # [D, FFN]
    w_u_t = (ffn_up_w * norm2_w[None, :]).T
    w_d_t = ffn_down_w.T                                 # [FFN, D]

    in_maps = []
    for r in range(NC):
        hsl = slice(r * HS, (r + 1) * HS)
        dsl = slice(r * DS, (r + 1) * DS)
        fsl = slice(r * FS, (r + 1) * FS)
        w3_r = np.concatenate(
            [w_in_t[:, hsl], w_ig_t[:, hsl], w_rg_t[:, hsl]], axis=1)
        wg_r = np.zeros((D, FSP), f)
        wg_r[:, :FS] = w_g_t[:, fsl]
        wu_r = np.zeros((D, FSP), f)
        wu_r[:, :FS] = w_u_t[:, fsl]
        wd_r = np.zeros((FSP, D), f)
        wd_r[:FS, :] = w_d_t[fsl, :]
        smalls_r = np.stack(
            [rec_lambda[hsl], rec_ig_b[hsl], rec_rg_b[hsl],
             np.broadcast_to(rec_h0[0, 0], (H,))[hsl]], axis=1)
        in_maps.append({
            "xt": xt_full,
            "xbf": np.ascontiguousarray(xt_full[dsl, :]),
            "w3": np.ascontiguousarray(w3_r.astype(NP_BF16)),
            "wro": np.ascontiguousarray(w_ro_t[:, dsl].astype(NP_BF16)),
            "wg": np.ascontiguousarray(wg_r.astype(NP_BF16)),
            "wu": np.ascontiguousarray(wu_r.astype(NP_BF16)),
            "wd": np.ascontiguousarray(wd_r.astype(NP_BF16)),
            "smalls": np.ascontiguousarray(smalls_r.astype(f)),
        })
    return in_maps


def run_on_device(inputs, trace=False, tmpdir=None):
    if "nc" not in _CACHE:
        _CACHE["nc"] = build_nc()
    nc = _CACHE["nc"]
    in_maps = _prep_inputs(inputs)
    res = run_bass_kernel_spmd(nc, in_maps, list(range(NC)),
                               trace=trace, tmpdir=tmpdir)
    shards = [np.asarray(res.results[r]["y"]) for r in range(NC)]
    yt = np.concatenate(shards, axis=0)                  # [D, BT]
    out = np.ascontiguousarray(yt.T).reshape(B, T, D).astype(np.float32)
    return out, res


def kernel(**inputs):
    out, _ = run_on_device(inputs, trace=False)
    return out


# revision 17
# speedup vs baseline: 1.0237x; 1.0119x over previous
"""Trainium2 Bass kernel for a Griffin-style ChimeraBlock:
   pre-norm RG-LRU recurrence branch + pre-norm SwiGLU FFN, B=2, T=2048,
   D=H=2048, FFN=5632, fp32 I/O.

Parallelization over 8 NeuronCores (tensor-parallel), v2 (pipelined):
  - recurrence hidden dim H sharded 8x (256/core); native DVE
    tensor_tensor_scan; FFN hidden sharded 8x (704 -> padded 768).
  - rmsnorm scale factors are applied AFTER the matmuls (per-token column
    scale), so matmuls never wait on the stats AllReduces.
  - hs sum-of-squares partials ride the hs AllGather as a 257th row
    (no separate AllReduce); xnew stats use small per-pair AllReduces.
  - the whole back half (rec_out -> xnew -> h2 -> AG -> FFN -> RS -> y)
    is pipelined per 2-chunk pair so PE never drains between phases.
  - xnew kept in SBUF (bf16); final y = xnew + ffn_red in f32.
Matmuls run in bf16 (fp32 accumulation in PSUM); scan state and the
a_t decay factors stay fp32 (bf16 a_t would drift on long-horizon
channels); most other element-wise tiles are bf16 for SBUF headroom.
"""

import sys

sys.path.insert(0, "/opt/trn_rl_repo")

import numpy as np
import ml_dtypes

import concourse.bass as bass
import concourse.mybir as mybir
import concourse.tile as tile
from concourse.tile_rust import add_dep_helper
from concourse import bacc
from concourse.bass_utils import run_bass_kernel_spmd

BF16 = mybir.dt.bfloat16
F32 = mybir.dt.float32
AF = mybir.ActivationFunctionType
OP = mybir.AluOpType

B, T, D = 2, 2048, 2048
H, FFN = 2048, 5632
NC = 8
HS = H // NC          # 256 hidden shard
DS = D // NC          # 256 d-model shard (output sharding)
FS = FFN // NC        # 704 ffn shard
FSP = 768             # ffn shard padded to a multiple of 128 (pad weights = 0)
BT = B * T            # 4096
CH = 512              # time-chunk (columns)
NCH = BT // CH        # 8 chunks
CPB = T // CH         # 4 chunks per batch element (scan resets at b boundary)
KD = D // 128         # 16 k-tiles when contracting over D
KH = H // 128         # 16 k-tiles when contracting over H
KF = FSP // 128       # 6 k-tiles when contracting over ffn shard
NP = NCH // 2         # 4 chunk-pairs
EPS = 1e-6
CCONST = 8.0

NP_BF16 = ml_dtypes.bfloat16


def _r128(ap):
    # [R, N] dram view -> [128, R//128, N] (partition, k-tile, col)
    return ap.rearrange("(k p) n -> p k n", p=128)


def build_nc(phases=7):
    nc = bacc.Bacc("TRN2", target_bir_lowering=False, debug=False, num_devices=NC)
    rg = [list(range(NC))]

    # ---------------- kernel I/O (per core) ----------------
    xt = nc.dram_tensor("xt", [D, BT], BF16, kind="ExternalInput")      # x^T replicated
    xbf = nc.dram_tensor("xbf", [DS, BT], BF16, kind="ExternalInput")   # bf16 x^T d-shard
    w3 = nc.dram_tensor("w3", [D, 3 * HS], BF16, kind="ExternalInput")  # in|ig|rg lhsT shard
    wro = nc.dram_tensor("wro", [H, DS], BF16, kind="ExternalInput")    # rec_out lhsT d-shard
    wg = nc.dram_tensor("wg", [D, FSP], BF16, kind="ExternalInput")
    wu = nc.dram_tensor("wu", [D, FSP], BF16, kind="ExternalInput")
    wd = nc.dram_tensor("wd", [FSP, D], BF16, kind="ExternalInput")
    # cols: 0 = rec_lambda, 1 = ig bias, 2 = rg bias, 3 = h0
    smalls = nc.dram_tensor("smalls", [HS, 4], F32, kind="ExternalInput")
    y = nc.dram_tensor("y", [DS, BT], F32, kind="ExternalOutput")

    with tile.TileContext(nc) as tc:
        with (
            tc.tile_pool(name="sb", bufs=2) as sb,
            tc.tile_pool(name="ps", bufs=2, space="PSUM") as ps,
            tc.tile_pool(name="dr", bufs=1, space="DRAM") as dr,
        ):
            build_body(nc, tc, sb, ps, dr, rg,
                       xt, xbf, w3, wro, wg, wu, wd, smalls, y, phases)
    nc.compile()
    return nc


def build_body(nc, tc, sb, ps, dr, rg, xt, xbf, w3, wro, wg, wu, wd, smalls, y,
               phases=7):
    def finish_early():
        fin = sb.tile([128, 4], F32, name="fin", tag="fin", bufs=1)
        nc.vector.memset(fin[:], 0.0)
        nc.sync.dma_start(out=y[0:128, 0:4], in_=fin[:])
    AG = "AllGather"
    AR = "AllReduce"
    RS = "ReduceScatter"

    dma = nc.sync.dma_start

    # ---------------- internal DRAM ----------------
    ar1_in = dr.tile([1, BT], BF16, name="ar1_in")
    ar1_out = dr.tile([1, BT], BF16, name="ar1_out", addr_space="Shared")
    ar3_in = [dr.tile([1, 2 * CH], BF16, name=f"ar3_in{j}") for j in range(NP)]
    ar3_out = [dr.tile([1, 2 * CH], BF16, name=f"ar3_out{j}", addr_space="Shared")
               for j in range(NP)]
    # hs AG payload: 256 data rows + 1 stats row (bf16 partial sum-of-squares)
    agin_hs = [dr.tile([HS + 1, 2 * CH], BF16, name=f"agin_hs{j}")
               for j in range(NP)]
    agout_hs = [dr.tile([NC * (HS + 1), 2 * CH], BF16, name=f"agout_hs{j}",
                        addr_space="Shared") for j in range(NP)]
    agin_h2 = [dr.tile([DS, 2 * CH], BF16, name=f"agin_h2{j}") for j in range(NP)]
    agout_h2 = [dr.tile([D, 2 * CH], BF16, name=f"agout_h2{j}", addr_space="Shared")
                for j in range(NP)]
    ffn_part = [dr.tile([D, 2 * CH], BF16, name=f"ffn_part{j}")
                for j in range(NP - 1)]
    ffn_red = [dr.tile([DS, 2 * CH], BF16, name=f"ffn_red{j}")
               for j in range(NP - 1)]
    # last pair: per-chunk RS so the tail only waits on a half-size collective
    ffn_part3 = [dr.tile([D, CH], BF16, name=f"ffn_part3_{h}") for h in range(2)]
    ffn_red3 = [dr.tile([DS, CH], BF16, name=f"ffn_red3_{h}") for h in range(2)]

    # ---------------- constants / small tensors ----------------
    ones_bf = sb.tile([128, 1], BF16, name="ones_bf", tag="ones", bufs=1)
    nc.vector.memset(ones_bf[:], 1.0)

    def const_tile(val, cname):
        t = sb.tile([128, 1], F32, name=cname, tag=cname, bufs=1)
        nc.vector.memset(t[:], val)
        return t

    # 32-row ones/zeros column for the K=8 stats reduction: rows 8-31 must be
    # explicit zeros (stale PE rows / SBUF garbage otherwise contaminate K<32
    # contractions, which always run as a 32-row hardware tile)
    ones8z = sb.tile([32, 1], BF16, name="ones8z", tag="ones8z", bufs=1)
    nc.vector.memset(ones8z[:], 0.0)
    nc.vector.memset(ones8z[0:8, :], 1.0)  # base partition 0, size 8

    c_ln8 = const_tile(1e-8, "c_ln8")       # Ln bias
    c_eps = const_tile(EPS, "c_eps")        # rmsnorm eps
    c_1eps = const_tile(1.0 + EPS, "c_1eps")  # 1 + eps for sqrt(1 - a^2 + eps)

    smalls_sb = sb.tile([128, 2, 4], F32, name="smalls_sb", tag="smalls", bufs=1)
    dma(out=smalls_sb[:], in_=smalls[:].rearrange("(a p) c -> p a c", p=128))
    sig_l = sb.tile([128, 2], F32, name="sig_l", tag="sig_l", bufs=1)
    nc.scalar.activation(sig_l[:], smalls_sb[:, :, 0], AF.Sigmoid)
    c8_sb = sb.tile([128, 2], F32, name="c8_sb", tag="c8", bufs=1)
    # log(sigmoid(lambda) + 1e-8)
    nc.scalar.activation(c8_sb[:], sig_l[:], AF.Ln, bias=c_ln8[:])
    # * C (in place via Copy with scale)
    nc.scalar.activation(c8_sb[:], c8_sb[:], AF.Copy, bias=0.0, scale=CCONST)

    # ---------------- weights ----------------
    # bigw slots rotate: w3, wg, wu early; wd reuses w3's slot after in-proj.
    w3_sb = sb.tile([128, KD, 3 * HS], BF16, name="w3_sb", tag="bigw", bufs=3)
    for q in range(4):
        dma(out=w3_sb[:, 4 * q:4 * (q + 1), :],
            in_=_r128(w3[:])[:, 4 * q:4 * (q + 1), :])
    # ---------------- phase 1: norm1 stats (partial over d-shard) ----------------
    for c in range(NCH):
        cs = slice(c * CH, (c + 1) * CH)
        xft = sb.tile([128, 2, CH], BF16, name=f"xft{c}", tag="xf", bufs=2)
        dma(out=xft[:], in_=_r128(xbf[:])[:, :, cs])
        xsq = sb.tile([128, 2, CH], BF16, name=f"xsq{c}", tag="sq", bufs=1)
        nc.scalar.activation(xsq[:], xft[:], AF.Square)
        psq = ps.tile([1, CH], F32, name=f"psq1_{c}", tag="psq", bufs=2)
        nc.tensor.matmul(psq[:], ones_bf[:], xsq[:, 0, :], start=True, stop=False)
        nc.tensor.matmul(psq[:], ones_bf[:], xsq[:, 1, :], start=False, stop=True)
        sqs = sb.tile([1, CH], BF16, name=f"sqs1_{c}", tag="sqs", bufs=2)
        nc.scalar.copy(sqs[:], psq[:])
        dma(out=ar1_in[0:1, cs], in_=sqs[:])
    nc.gpsimd.collective_compute(AR, OP.add, replica_groups=rg,
                                 ins=[ar1_in[:]], outs=[ar1_out[:]])

    if phases < 2:
        finish_early()
        return

    # ---------------- phase 2: in-proj + gates + scan ----------------
    late_w = {}

    def load_late_weights():
        late_w["wro"] = sb.tile([128, KH, DS], BF16, name="wro_sb", tag="wro",
                                bufs=1)
        dma(out=late_w["wro"][:], in_=_r128(wro[:]))
        late_w["wg"] = sb.tile([128, KD, FSP], BF16, name="wg_sb", tag="bigw",
                               bufs=3)
        dma(out=late_w["wg"][:], in_=_r128(wg[:]))
        late_w["wu"] = sb.tile([128, KD, FSP], BF16, name="wu_sb", tag="bigw",
                               bufs=3)
        dma(out=late_w["wu"][:], in_=_r128(wu[:]))

    hst_prev = None
    for c in range(NCH):
        cs = slice(c * CH, (c + 1) * CH)
        j, jj = c // 2, c % 2

        xc = sb.tile([128, KD, CH], BF16, name=f"xc{c}", tag="stream", bufs=2)
        dma(out=xc[:], in_=_r128(xt[:])[:, :, cs])

        # matmuls first: they do not depend on AR1
        pst = {}
        for g in range(6):  # (proj, m): 0,1 x_proj; 2,3 ig; 4,5 rg
            p = ps.tile([128, CH], F32, name=f"pp{c}_{g}", tag="mm", bufs=6)
            for k in range(KD):
                nc.tensor.matmul(
                    p[:], w3_sb[:, k, g * 128:(g + 1) * 128], xc[:, k, :],
                    start=(k == 0), stop=(k == KD - 1))
            pst[g] = p

        # raw psum -> SBUF immediately (ACT copy): frees PSUM without AR1
        zt = sb.tile([128, 6, CH], BF16, name=f"z{c}", tag="zgu", bufs=2)
        for g in range(6):
            nc.scalar.copy(zt[:, g, :], pst[g][:])

        # inv_rms1 for this chunk (scale applied after the matmul)
        arc = sb.tile([1, CH], BF16, name=f"arc{c}", tag="arc", bufs=1)
        dma(out=arc[:], in_=ar1_out[0:1, cs])
        rsq = sb.tile([1, CH], F32, name=f"rsq1_{c}", tag="srow", bufs=2)
        nc.scalar.activation(rsq[:], arc[:], AF.Sqrt,
                             bias=c_eps[:1, :], scale=1.0 / D)
        rsqb = sb.tile([1, CH], BF16, name=f"rsqb1_{c}", tag="rsqb", bufs=1)
        with nc.allow_low_precision(reason="bf16 per-token inv_rms scale"):
            nc.vector.reciprocal(rsqb[:], rsq[:])
        invc = sb.tile([128, CH], BF16, name=f"invc1_{c}", tag="invc", bufs=4)
        nc.gpsimd.partition_broadcast(invc[:], rsqb[:])

        for g in range(6):
            nc.vector.tensor_tensor(zt[:, g, :], zt[:, g, :], invc[:], op=OP.mult)

        if c == 1:
            load_late_weights()

        hst = sb.tile([128, 2, CH], BF16, name=f"hst{c}", tag="hs", bufs=3)
        # ACT ops grouped by function to limit activation-table swaps
        it = {}
        rt = {}
        for m in range(2):
            it[m] = sb.tile([128, CH], BF16, name=f"it{c}_{m}", tag=f"it{m}", bufs=2)
            nc.scalar.activation(it[m][:], zt[:, 2 + m, :], AF.Sigmoid,
                                 bias=smalls_sb[:, m, 1:2])
            rt[m] = sb.tile([128, CH], BF16, name=f"rt{c}_{m}", tag=f"rt{m}", bufs=2)
            nc.scalar.activation(rt[m][:], zt[:, 4 + m, :], AF.Sigmoid,
                                 bias=smalls_sb[:, m, 2:3])
        at = sb.tile([128, 2, CH], F32, name=f"at{c}", tag="at", bufs=1)
        for m in range(2):
            # la = r * (C * log_a)  (in place over rt)
            nc.vector.tensor_scalar_mul(rt[m][:], rt[m][:], c8_sb[:, m:m + 1])
            nc.scalar.activation(at[:, m, :], rt[m][:], AF.Exp)
        na = {}
        for m in range(2):
            # f32: 1 - a^2 suffers catastrophic bf16 quantization near a = 1
            na[m] = sb.tile([128, CH], F32, name=f"na{c}_{m}", tag=f"na{m}", bufs=2)
            nc.vector.scalar_tensor_tensor(na[m][:], at[:, m, :], -1.0, at[:, m, :],
                                           op0=OP.mult, op1=OP.mult)
        for m in range(2):
            nc.scalar.activation(na[m][:], na[m][:], AF.Sqrt, bias=c_1eps[:])
        for m in range(2):
            # g = sq * (i * x_proj)   (in place over the x_proj z-slice)
            nc.vector.tensor_tensor(zt[:, m, :], it[m][:], zt[:, m, :], op=OP.mult)
            nc.vector.tensor_tensor(zt[:, m, :], na[m][:], zt[:, m, :], op=OP.mult)
            if c % CPB == 0:
                init = smalls_sb[:, m, 3:4]
            else:
                init = hst_prev[:, m, CH - 1:CH]
            nc.vector.tensor_tensor_scan(hst[:, m, :], at[:, m, :], zt[:, m, :],
                                         init, op0=OP.mult, op1=OP.add)
        hst_prev = hst

        # partial sumsq of hs over the h-shard -> stats row of the AG payload
        hsq = sb.tile([128, 2, CH], BF16, name=f"hsq{c}", tag="sq", bufs=1)
        nc.scalar.activation(hsq[:], hst[:], AF.Square)
        psq2 = ps.tile([1, CH], F32, name=f"psq2_{c}", tag="psq", bufs=2)
        nc.tensor.matmul(psq2[:], ones_bf[:], hsq[:, 0, :], start=True, stop=False)
        nc.tensor.matmul(psq2[:], ones_bf[:], hsq[:, 1, :], start=False, stop=True)
        sqs2 = sb.tile([1, CH], BF16, name=f"sqs2_{c}", tag="sqs", bufs=2)
        nc.scalar.copy(sqs2[:], psq2[:])
        jsl = slice(jj * CH, (jj + 1) * CH)
        dma(out=agin_hs[j][0:HS, jsl].rearrange("(a p) n -> p a n", p=128),
            in_=hst[:])
        dma(out=agin_hs[j][HS:HS + 1, jsl], in_=sqs2[:])
        if jj == 1:
            nc.gpsimd.collective_compute(AG, OP.bypass, replica_groups=rg,
                                         ins=[agin_hs[j][:]], outs=[agout_hs[j][:]])

    if phases < 4:
        finish_early()
        return

    # ------- merged back half: per pair j, rec_out/residual/h2/AG for pair j
    # ------- then FFN + RS + final residual for pair j-1 (1-pair SW pipeline)
    wd_sb = sb.tile([128, KF, D], BF16, name="wd_sb", tag="bigw", bufs=3)
    dma(out=wd_sb[:], in_=_r128(wd[:]))

    xnt_t = {}
    last_insts = {}

    def phase4_chunk(c, invc2_pair):
        cs = slice(c * CH, (c + 1) * CH)
        j, jj = c // 2, c % 2
        # agout_hs rows: q*(HS+1) + [0, 256) data, q*(HS+1)+256 stats
        hs_view = agout_hs[j][:].rearrange("(q x) n -> q x n", x=HS + 1)

        if jj == 0:
            # per-pair: sum the 8 bf16 stats partials -> invc2 over both chunks
            st2 = sb.tile([32, 1, 2 * CH], BF16, name=f"st2_{j}", tag="st2", bufs=1)
            nc.vector.memset(st2[:], 0.0)
            dma(out=st2[0:8, :, :], in_=hs_view[:, HS:HS + 1, :])
            rsq2 = sb.tile([1, 2 * CH], F32, name=f"rsq2_{j}", tag="srow", bufs=2)
            for half in range(2):
                hsl = slice(half * CH, (half + 1) * CH)
                pst2 = ps.tile([1, CH], F32, name=f"ps2_{j}_{half}", tag="psq",
                               bufs=2)
                nc.tensor.matmul(pst2[:], ones8z[:], st2[:, 0, hsl],
                                 start=True, stop=True)
                nc.scalar.activation(rsq2[:, hsl], pst2[:], AF.Sqrt,
                                     bias=c_eps[:1, :], scale=1.0 / H)
            rsqb2 = sb.tile([1, 2 * CH], BF16, name=f"rsqb2_{j}", tag="rsqb2",
                            bufs=1)
            with nc.allow_low_precision(reason="bf16 per-token inv_rms scale"):
                nc.vector.reciprocal(rsqb2[:], rsq2[:])
            inv2 = {}
            for jj2 in range(2):
                inv2[jj2] = sb.tile([128, CH], BF16, name=f"invc2_{j}_{jj2}",
                                    tag="invc", bufs=4)
                nc.gpsimd.partition_broadcast(
                    inv2[jj2][:], rsqb2[0:1, jj2 * CH:(jj2 + 1) * CH])
            invc2_pair = inv2

        hstm = sb.tile([128, KH // 2, 2, CH], BF16, name=f"hstm{c}", tag="stream",
                       bufs=2)
        for h in range(2):
            hd = dma(out=hstm[:, :, h, :],
                     in_=hs_view[:, 0:HS, :]
                     .rearrange("q (h p) n -> p q h n", p=128)
                     [:, :, h, jj * CH:(jj + 1) * CH])
            last_insts["hstm"] = hd
        xft2 = sb.tile([128, 2, CH], BF16, name=f"xfr{c}", tag="xf", bufs=2)
        dma(out=xft2[:], in_=_r128(xbf[:])[:, :, cs])

        xnt = sb.tile([128, 2, CH], BF16, name=f"xnt{c}", tag="xnt", bufs=8)
        xnt_t[c] = xnt
        for m in range(2):
            pro = ps.tile([128, CH], F32, name=f"pro{c}_{m}", tag="mm", bufs=6)
            for kt in range(KH):
                nc.tensor.matmul(pro[:], late_w["wro"][:, kt, m * 128:(m + 1) * 128],
                                 hstm[:, kt // 2, kt % 2, :],
                                 start=(kt == 0), stop=(kt == KH - 1))
            nc.vector.tensor_tensor(xnt[:, m, :], pro[:], invc2_pair[jj][:],
                                    op=OP.mult)
            nc.vector.tensor_tensor(xnt[:, m, :], xnt[:, m, :], xft2[:, m, :],
                                    op=OP.add)

        # xnew stats for this chunk -> per-pair AllReduce
        xnq = sb.tile([128, 2, CH], BF16, name=f"xnq{c}", tag="sq", bufs=1)
        nc.scalar.activation(xnq[:], xnt[:], AF.Square)
        psq3 = ps.tile([1, CH], F32, name=f"psq3_{c}", tag="psq", bufs=2)
        nc.tensor.matmul(psq3[:], ones_bf[:], xnq[:, 0, :], start=True, stop=False)
        nc.tensor.matmul(psq3[:], ones_bf[:], xnq[:, 1, :], start=False, stop=True)
        sqs3 = sb.tile([1, CH], BF16, name=f"sqs3_{c}", tag="sqs", bufs=2)
        nc.scalar.copy(sqs3[:], psq3[:])
        dma(out=ar3_in[j][0:1, jj * CH:(jj + 1) * CH], in_=sqs3[:])

        if jj == 1:
            nc.gpsimd.collective_compute(AR, OP.add, replica_groups=rg,
                                         ins=[ar3_in[j][:]], outs=[ar3_out[j][:]])
            # h2 = xnew * invc3 for both chunks of the pair, then AG
            arc3 = sb.tile([1, 2 * CH], BF16, name=f"arc3_{j}", tag="arc3", bufs=1)
            nc.gpsimd.dma_start(out=arc3[:], in_=ar3_out[j][:])
            rsq3 = sb.tile([1, 2 * CH], F32, name=f"rsq3_{j}", tag="srow", bufs=2)
            nc.scalar.activation(rsq3[:], arc3[:], AF.Sqrt,
                                 bias=c_eps[:1, :], scale=1.0 / D)
            rsqb3 = sb.tile([1, 2 * CH], BF16, name=f"rsqb3_{j}", tag="rsqb2",
                            bufs=1)
            with nc.allow_low_precision(reason="bf16 per-token inv_rms scale"):
                nc.vector.reciprocal(rsqb3[:], rsq3[:])
            for cc in (c - 1, c):
                ccj = cc % 2
                inv3 = sb.tile([128, CH], BF16, name=f"invc3_{j}_{ccj}",
                               tag="invc", bufs=4)
                nc.gpsimd.partition_broadcast(
                    inv3[:], rsqb3[0:1, ccj * CH:(ccj + 1) * CH])
                h2t = sb.tile([128, 2, CH], BF16, name=f"h2t{j}_{ccj}", tag="h2t",
                              bufs=1)
                for m in range(2):
                    nc.vector.tensor_tensor(h2t[:, m, :], xnt_t[cc][:, m, :],
                                            inv3[:], op=OP.mult)
                dma(out=agin_h2[j][:, ccj * CH:(ccj + 1) * CH]
                    .rearrange("(a p) n -> p a n", p=128), in_=h2t[:])
            nc.gpsimd.collective_compute(AG, OP.bypass, replica_groups=rg,
                                         ins=[agin_h2[j][:]], outs=[agout_h2[j][:]])
        return invc2_pair

    def phase6_chunk(c):
        j, jj = c // 2, c % 2
        h2s = sb.tile([128, KD, CH], BF16, name=f"h2s{c}", tag="stream", bufs=2)
        hh = dma(out=h2s[:], in_=_r128(agout_h2[j][:])[:, :, jj * CH:(jj + 1) * CH])
        if last_insts.get("hstm") is not None:
            # shared stream slots: never let an FFN input load jump ahead of
            # the remaining rec_out input loads (slot-allocation deadlock)
            add_dep_helper(hh.ins, last_insts["hstm"].ins, sync=False,
                           reason="stream slot order: h2s after all hstm")
        gu = sb.tile([128, KF, CH], BF16, name=f"gu{c}", tag="zgu", bufs=2)
        for m in range(KF):
            psg = ps.tile([128, CH], F32, name=f"pg{c}_{m}", tag="mm", bufs=6)
            for k in range(KD):
                nc.tensor.matmul(psg[:], late_w["wg"][:, k, m * 128:(m + 1) * 128],
                                 h2s[:, k, :],
                                 start=(k == 0), stop=(k == KD - 1))
            gs = sb.tile([128, CH], BF16, name=f"gs{c}_{m}", tag="gsil", bufs=2)
            nc.scalar.activation(gs[:], psg[:], AF.Silu)
            psu = ps.tile([128, CH], F32, name=f"pu{c}_{m}", tag="mm", bufs=6)
            for k in range(KD):
                nc.tensor.matmul(psu[:], late_w["wu"][:, k, m * 128:(m + 1) * 128],
                                 h2s[:, k, :],
                                 start=(k == 0), stop=(k == KD - 1))
            nc.vector.tensor_tensor(gu[:, m, :], psu[:], gs[:], op=OP.mult)
        for m in range(KD):
            psd = ps.tile([128, CH], F32, name=f"pd{c}_{m}", tag="mm", bufs=6)
            for k in range(KF):
                nc.tensor.matmul(psd[:], wd_sb[:, k, m * 128:(m + 1) * 128],
                                 gu[:, k, :],
                                 start=(k == 0), stop=(k == KF - 1))
            dst = sb.tile([128, CH], BF16, name=f"dst{c}_{m}", tag="dstage", bufs=2)
            if m % 2 == 0:
                nc.vector.tensor_copy(dst[:], psd[:])
            else:
                nc.scalar.copy(dst[:], psd[:])
            if j == NP - 1:
                dma(out=ffn_part3[jj][m * 128:(m + 1) * 128, :], in_=dst[:])
            else:
                dma(out=ffn_part[j][m * 128:(m + 1) * 128, jj * CH:(jj + 1) * CH],
                    in_=dst[:])
        if j == NP - 1:
            nc.gpsimd.collective_compute(RS, OP.add, replica_groups=rg,
                                         ins=[ffn_part3[jj][:]],
                                         outs=[ffn_red3[jj][:]])
        elif jj == 1:
            nc.gpsimd.collective_compute(RS, OP.add, replica_groups=rg,
                                         ins=[ffn_part[j][:]], outs=[ffn_red[j][:]])

    def phase7_chunk(c):
        cs = slice(c * CH, (c + 1) * CH)
        j, jj = c // 2, c % 2
        for m in range(2):
            frt = sb.tile([128, CH], BF16, name=f"frt{c}_{m}", tag="frt", bufs=2)
            if j == NP - 1:
                dma(out=frt[:], in_=_r128(ffn_red3[jj][:])[:, m, :])
            else:
                dma(out=frt[:],
                    in_=_r128(ffn_red[j][:])[:, m, jj * CH:(jj + 1) * CH])
            yt = sb.tile([128, CH], F32, name=f"yt{c}_{m}", tag="yt", bufs=2)
            nc.vector.tensor_tensor(yt[:], xnt_t[c][:, m, :], frt[:], op=OP.add)
            dma(out=_r128(y[:])[:, m, cs], in_=yt[:])

    for j in range(NP):
        invc2_pair = phase4_chunk(2 * j, None)
        phase4_chunk(2 * j + 1, invc2_pair)

    if phases >= 6:
        for j in range(NP):
            phase6_chunk(2 * j)
            phase6_chunk(2 * j + 1)
            if phases >= 7 and j >= 1:
                phase7_chunk(2 * (j - 1))
                phase7_chunk(2 * (j - 1) + 1)
        if phases >= 7:
            phase7_chunk(2 * (NP - 1))
            phase7_chunk(2 * (NP - 1) + 1)

    if phases < 6:
        # debug: dump xnew to y
        for c in range(NCH):
            cs = slice(c * CH, (c + 1) * CH)
            ytd = sb.tile([128, 2, CH], F32, name=f"ytd{c}", tag="yt", bufs=2)
            nc.vector.tensor_copy(ytd[:], xnt_t[c][:])
            dma(out=_r128(y[:])[:, :, cs], in_=ytd[:])
        return
    if phases < 7:
        finish_early()
        return


_CACHE = {}


def _prep_inputs(inputs):
    f = np.float32
    x = np.asarray(inputs["x"], f)                       # [B, T, D]
    norm1_w = np.asarray(inputs["norm1_w"], f)
    rec_in_w = np.asarray(inputs["rec_in_w"], f)         # [H, D]
    rec_ig_w = np.asarray(inputs["rec_ig_w"], f)
    rec_ig_b = np.asarray(inputs["rec_ig_b"], f)
    rec_rg_w = np.asarray(inputs["rec_rg_w"], f)
    rec_rg_b = np.asarray(inputs["rec_rg_b"], f)
    rec_lambda = np.asarray(inputs["rec_lambda"], f)
    rec_out_w = np.asarray(inputs["rec_out_w"], f)       # [D, H]
    rec_h0 = np.asarray(inputs["rec_h0"], f)             # [1, 1, H]
    rec_norm_w = np.asarray(inputs["rec_norm_w"], f)
    norm2_w = np.asarray(inputs["norm2_w"], f)
    ffn_gate_w = np.asarray(inputs["ffn_gate_w"], f)     # [FFN, D]
    ffn_up_w = np.asarray(inputs["ffn_up_w"], f)
    ffn_down_w = np.asarray(inputs["ffn_down_w"], f)     # [D, FFN]

    xt_full = np.ascontiguousarray(
        x.reshape(BT, D).T.astype(NP_BF16))              # [D, BT]

    # fold norm gains into adjacent weights; transpose into lhsT layouts
    w_in_t = (rec_in_w * norm1_w[None, :]).T             # [D, H]
    w_ig_t = (rec_ig_w * norm1_w[None, :]).T
    w_rg_t = (rec_rg_w * norm1_w[None, :]).T
    w_ro_t = (rec_out_w * rec_norm_w[None, :]).T         # [H, D]
    w_g_t = (ffn_gate_w * norm2_w[None, :]).T            # [D, FFN]
    w_u_t = (ffn_up_w * norm2_w[None, :]).T
    w_d_t = ffn_down_w.T                                 # [FFN, D]

    in_maps = []
    for r in range(NC):
        hsl = slice(r * HS, (r + 1) * HS)
        dsl = slice(r * DS, (r + 1) * DS)
        fsl = slice(r * FS, (r + 1) * FS)
        w3_r = np.concatenate(
            [w_in_t[:, hsl], w_ig_t[:, hsl], w_rg_t[:, hsl]], axis=1)
        wg_r = np.zeros((D, FSP), f)
        wg_r[:, :FS] = w_g_t[:, fsl]
        wu_r = np.zeros((D, FSP), f)
        wu_r[:, :FS] = w_u_t[:, fsl]
        wd_r = np.zeros((FSP, D), f)
        wd_r[:FS, :] = w_d_t[fsl, :]
        smalls_r = np.stack(
            [rec_lambda[hsl], rec_ig_b[hsl], rec_rg_b[hsl],
             np.broadcast_to(rec_h0[0, 0], (H,))[hsl]], axis=1)
        in_maps.append({
            "xt": xt_full,
            "xbf": np.ascontiguousarray(xt_full[dsl, :]),
            "w3": np.ascontiguousarray(w3_r.astype(NP_BF16)),
            "wro": np.ascontiguousarray(w_ro_t[:, dsl].astype(NP_BF16)),
            "wg": np.ascontiguousarray(wg_r.astype(NP_BF16)),
            "wu": np.ascontiguousarray(wu_r.astype(NP_BF16)),
            "wd": np.ascontiguousarray(wd_r.astype(NP_BF16)),
            "smalls": np.ascontiguousarray(smalls_r.astype(f)),
        })
    return in_maps


def run_on_device(inputs, trace=False, tmpdir=None):
    if "nc" not in _CACHE:
        _CACHE["nc"] = build_nc()
    nc = _CACHE["nc"]
    in_maps = _prep_inputs(inputs)
    res = run_bass_kernel_spmd(nc, in_maps, list(range(NC)),
                               trace=trace, tmpdir=tmpdir)
    shards = [np.asarray(res.results[r]["y"]) for r in range(NC)]
    yt = np.concatenate(shards, axis=0)                  # [D, BT]
    out = np.ascontiguousarray(yt.T).reshape(B, T, D).astype(np.float32)
    return out, res


def kernel(**inputs):
    out, _ = run_on_device(inputs, trace=False)
    return out
